# revision 84
# baseline (speedup 1.0000x reference)
"""3-layer GAT on 8 Trainium2 NeuronCores (Bass/Tile).

Edge-sharded by destination range:
  - Nodes split into 8 contiguous ranges (one per core); each core owns the
    softmax + aggregation for its destination nodes.
  - Layer 1's per-node table [h | a_src] is computed LOCALLY on every core
    from a replicated copy of x (no collective): x is an input, so each core
    can build the full 50176-row table with 392 small bf16 matmuls, which is
    much cheaper than the 38MB AllGather it replaces.  Per-core a_dst rows
    come from a second tiny matmul pass over the core's own x shard.
  - For layers 2/3 the aggregation output is only known by the dst-owning
    core, so tables are AllGathered -- but only the used columns (264 of
    384 / 48 of 128) with a strided output AP that lands the rows in the
    768B/256B-stride layout dma_gather needs.
  - Edges (with self loops) are bucketed per core into 128-dst tiles x
    128-edge chunks; chunk structure (incl. lo/hi int16-index table halves)
    is made identical across cores so one SPMD instruction stream fits all.
  - Per 8-chunk super-batch the kernel dma_gathers source rows + dest
    attention rows, computes w = exp(leaky_relu(a_src+a_dst)) (softmax
    shift-invariance removes the segment-max pass at these value ranges),
    scales messages by w, and segment-sums with matmuls against one-hot
    membership matrices, keeping numerator and denominator together in
    PSUM.  The per-tile epilogue divides, applies bias/relu, and feeds the
    next layer's matmul whose rhs [W | W@att_src | W@att_dst] also emits
    the next attention scores.
"""

import numpy as np
import ml_dtypes

N = 50000
E = 800000
IN_C = 128
HID = 32
OUT_C = 40
HEADS = 8
NEG_SLOPE = 0.2
NCORES = 8

_BF16 = ml_dtypes.bfloat16

KSUP = 8  # chunks per gather super-batch (1024-idx dma_gather limit)
SB = KSUP * 8  # int16 idx cols per sup for one index stream
BLK = 2 * SB + 2 * KSUP  # per-sup cols: sidx | didx | seg bytes


def _cmajor_perm(heads, ch):
    f_new = np.arange(heads * ch)
    return (f_new % heads) * ch + f_new // heads  # perm[new] = old


def _attn_cols(w, att):
    heads, ch = att.shape
    return np.einsum("khc,hc->kh", w.reshape(-1, heads, ch), att).astype(np.float32)


def _prep_weights(W1, as1, ad1, b1, W2, as2, ad2, b2, W3, as3, ad3, b3):
    W1 = np.asarray(W1, np.float32)
    W2 = np.asarray(W2, np.float32)
    W3 = np.asarray(W3, np.float32)
    perm = _cmajor_perm(HEADS, HID)

    rhs1 = np.concatenate(
        [W1[:, perm], _attn_cols(W1, np.asarray(as1, np.float32))],
        axis=1).astype(_BF16)
    rhs1d = _attn_cols(W1, np.asarray(ad1, np.float32)).astype(_BF16)
    W2r = W2[perm, :]
    rhs2 = np.concatenate(
        [W2r[:, perm], _attn_cols(W2r, np.asarray(as2, np.float32)),
         _attn_cols(W2r, np.asarray(ad2, np.float32))], axis=1).astype(_BF16)
    W3r = W3[perm, :]
    as3p = (W3r @ np.asarray(as3, np.float32)[0]).reshape(-1, 1)
    ad3p = (W3r @ np.asarray(ad3, np.float32)[0]).reshape(-1, 1)
    rhs3 = np.concatenate([W3r, as3p, ad3p], axis=1).astype(_BF16)

    def bcast(b):
        return np.tile(np.asarray(b, np.float32)[None, :], (128, 1))

    return (rhs1, rhs1d, rhs2, rhs3,
            bcast(np.asarray(b1, np.float32)[perm]),
            bcast(np.asarray(b2, np.float32)[perm]),
            bcast(np.asarray(b3, np.float32)))


def _prep_graph(edge_index):
    """Slot edges into the SPMD-uniform (tile, section, chunk) grid."""
    src = np.concatenate([edge_index[0], np.arange(N)]).astype(np.int64)
    dst = np.concatenate([edge_index[1], np.arange(N)]).astype(np.int64)

    npc = N // NCORES
    ntiles = (npc + 127) // 128
    nmax = ntiles * 128
    half = (NCORES // 2) * nmax

    core_of = dst // npc
    d_loc = dst - core_of * npc
    tile_of = d_loc // 128
    s_core = src // npc
    s_row = s_core * nmax + (src - s_core * npc)  # table row of src
    is_hi = s_row >= half

    cnt = np.zeros((NCORES, ntiles, 2), np.int64)
    np.add.at(cnt, (core_of, tile_of, is_hi.astype(np.int64)), 1)
    sec_cpt = np.ceil(cnt / 128).astype(np.int64).max(axis=0)  # [ntiles, 2]
    sec_cpt[:, 0] = np.maximum(sec_cpt[:, 0], 1)

    total = int(sec_cpt.sum())
    pad = (-total) % KSUP
    sec_cpt[-1, 1] += pad
    total += pad
    nsup = total // KSUP

    tile_of_chunk = []
    tag_of_chunk = []
    for t in range(ntiles):
        tile_of_chunk += [t] * int(sec_cpt[t, 0] + sec_cpt[t, 1])
        tag_of_chunk += [0] * int(sec_cpt[t, 0]) + [1] * int(sec_cpt[t, 1])
    tile_of_chunk = np.array(tile_of_chunk)
    tag_of_chunk = np.array(tag_of_chunk)
    sec_base = np.zeros((ntiles, 2), np.int64)
    sec_base.ravel()[1:] = np.cumsum(sec_cpt.ravel())[:-1]

    # combined per-sup stream: [sidx | didx | seg-bytes] int16 cols
    idxc = np.zeros((NCORES, 128, nsup, BLK), np.int16)
    seg = np.full((NCORES, nsup, 128, KSUP), 255.0, np.float32)

    order = np.lexsort((src, is_hi, tile_of, core_of))
    src_o = s_row[order]
    dst_o = d_loc[order]
    core_o = core_of[order]
    tile_o = tile_of[order]
    hi_o = is_hi[order]

    for k in range(NCORES):
        m = core_o == k
        t = tile_o[m]
        hi = hi_o[m].astype(np.int64)
        sr = src_o[m] - hi * half
        dl = dst_o[m]
        key = t * 2 + hi
        cnts = np.bincount(key, minlength=ntiles * 2)
        st = np.zeros(ntiles * 2, np.int64)
        st[1:] = np.cumsum(cnts)[:-1]
        pos = np.arange(len(t)) - st[key]
        q = sec_base[t, hi] + pos // 128
        p = pos % 128
        colsup = q // KSUP
        col = (q % KSUP) * 8 + p // 16
        row = p % 16
        # a_dst table rows are p-major (row = (d%128)*ntiles + d//128) so
        # the SBUF->DRAM flush is 128 contiguous runs instead of 6k tiny ones
        dlp = (dl % 128) * ntiles + dl // 128
        for c in range(8):
            idxc[k, row + 16 * c, colsup, col] = sr
            idxc[k, row + 16 * c, colsup, SB + col] = dlp
        seg[k, q // KSUP, p, q % KSUP] = (dl % 128).astype(np.float32)
    for k in range(NCORES):
        idxc[k, :, :, 2 * SB:] = np.ascontiguousarray(
            seg[k].transpose(1, 0, 2)).view(np.int16).reshape(
            128, nsup, 2 * KSUP)

    runs = []  # (sup, chunk_lo, chunk_hi, tag)
    for s in range(nsup):
        q0 = s * KSUP
        r0 = q0
        for q in range(q0 + 1, q0 + KSUP + 1):
            if q == q0 + KSUP or tag_of_chunk[q] != tag_of_chunk[r0]:
                runs.append((s, r0, q, int(tag_of_chunk[r0])))
                r0 = q

    return dict(
        idxc=idxc,
        tile_of_chunk=tile_of_chunk, runs=runs, nsup=nsup, total=total,
        ntiles=ntiles, nmax=nmax, npc=npc, half=half,
    )


def _build_bass(g, repeat=1):
    import os
    import concourse.bacc as bacc
    import concourse.mybir as mybir
    import concourse.tile as tile
    from concourse.masks import make_identity

    dt = mybir.dt
    Alu = mybir.AluOpType
    Act = mybir.ActivationFunctionType

    ntiles, nmax, nsup, total = g["ntiles"], g["nmax"], g["nsup"], g["total"]
    half = g["half"]
    tile_of_chunk = g["tile_of_chunk"]
    H2 = HEADS * HID  # 256
    GW = H2 + HEADS  # 264 useful table cols: h + a_src
    PACK = GW + HEADS  # 272 psum width in epilogue: h + a_src + a_dst
    TW = 384  # gather-table row width (768B)
    TW3 = 128  # layer-3 / a_dst table row width (256B)
    GW3 = OUT_C + 1  # 41
    LW3 = 48  # AllGathered cols for layer-3 table
    NTT = NCORES * ntiles  # total table tiles

    first_chunk = {}
    last_chunk = {}
    for q, t in enumerate(tile_of_chunk):
        first_chunk.setdefault(int(t), q)
        last_chunk[int(t)] = q
    runs_by_sup = {}
    for (s, a, b, tag) in g["runs"]:
        runs_by_sup.setdefault(s, []).append((a, b, tag))

    nphase = int(os.environ.get("GAT_PHASES", "3"))
    ag_wide = int(os.environ.get("GAT_AG_WIDE", "0"))
    # timing-only knock-outs (break correctness; for bottleneck isolation)
    no_srcg = int(os.environ.get("GAT_NO_SRCG", "0"))
    no_adst = int(os.environ.get("GAT_NO_ADST", "0"))
    no_vec = int(os.environ.get("GAT_NO_VEC", "0"))
    no_mm = int(os.environ.get("GAT_NO_MM", "0"))
    no_epil = int(os.environ.get("GAT_NO_EPIL", "0"))
    gbufs = int(os.environ.get("GAT_GBUFS", "8"))
    preload_idx = int(os.environ.get("GAT_PRELOAD_IDX", "1"))

    nc = bacc.Bacc("TRN2", target_bir_lowering=False, debug=False,
                   num_devices=NCORES, num_swdge_queues=4)

    xT = nc.dram_tensor("xT", [IN_C, NCORES * nmax], dt.bfloat16,
                        kind="ExternalInput")
    x_own = nc.dram_tensor("x_own", [IN_C, nmax], dt.bfloat16,
                           kind="ExternalInput")
    rhs1 = nc.dram_tensor("rhs1", [IN_C, GW], dt.bfloat16,
                          kind="ExternalInput")
    rhs1d = nc.dram_tensor("rhs1d", [IN_C, HEADS], dt.bfloat16,
                           kind="ExternalInput")
    rhs2 = nc.dram_tensor("rhs2", [H2, PACK], dt.bfloat16,
                          kind="ExternalInput")
    rhs3 = nc.dram_tensor("rhs3", [H2, OUT_C + 2], dt.bfloat16,
                          kind="ExternalInput")
    b1r = nc.dram_tensor("b1r", [128, H2], dt.float32, kind="ExternalInput")
    b2r = nc.dram_tensor("b2r", [128, H2], dt.float32, kind="ExternalInput")
    b3r = nc.dram_tensor("b3r", [128, OUT_C], dt.float32, kind="ExternalInput")
    iota = nc.dram_tensor("iota", [128, 128], dt.bfloat16, kind="ExternalInput")
    idxs_in = nc.dram_tensor("idxs", [128, nsup * BLK], dt.int16,
                             kind="ExternalInput")
    out = nc.dram_tensor("out", [nmax, OUT_C], dt.float32,
                         kind="ExternalOutput")

    with tile.TileContext(nc) as tc:
        with (
            tc.tile_pool(name="const", bufs=1) as constp,
            tc.tile_pool(name="sbuf", bufs=6) as sbuf,
            tc.tile_pool(name="gbuf", bufs=gbufs) as gbuf,
            tc.tile_pool(name="mbuf", bufs=6) as mbuf,
            tc.tile_pool(name="epil", bufs=2) as epil,
            tc.tile_pool(name="psum_seg", bufs=3, space="PSUM") as psum_seg,
            tc.tile_pool(name="psum_h", bufs=3, space="PSUM") as psum_h,
            tc.tile_pool(name="psum_tp", bufs=2, space="PSUM") as psum_tp,
            tc.tile_pool(name="dram", bufs=1, space="DRAM") as dram,
        ):
            # ---- constants ----
            rhs1_s = constp.tile([IN_C, GW], dt.bfloat16)
            nc.sync.dma_start(out=rhs1_s[:], in_=rhs1[:])
            rhs1d_s = constp.tile([IN_C, HEADS], dt.bfloat16)
            nc.sync.dma_start(out=rhs1d_s[:], in_=rhs1d[:])
            rhs2_s = constp.tile([128, 2 * PACK], dt.bfloat16)
            nc.sync.dma_start(
                out=rhs2_s[:].rearrange("p (k f) -> p k f", k=2),
                in_=rhs2[:].rearrange("(k p) f -> p k f", p=128))
            rhs3_s = constp.tile([128, 2 * (OUT_C + 2)], dt.bfloat16)
            nc.sync.dma_start(
                out=rhs3_s[:].rearrange("p (k f) -> p k f", k=2),
                in_=rhs3[:].rearrange("(k p) f -> p k f", p=128))
            b1_s = constp.tile([128, H2], dt.float32)
            nc.sync.dma_start(out=b1_s[:], in_=b1r[:])
            b2_s = constp.tile([128, H2], dt.float32)
            nc.sync.dma_start(out=b2_s[:], in_=b2r[:])
            b3_s = constp.tile([128, OUT_C], dt.float32)
            nc.sync.dma_start(out=b3_s[:], in_=b3r[:])
            iota_s = constp.tile([128, 128], dt.bfloat16)
            nc.sync.dma_start(out=iota_s[:], in_=iota[:])
            ident = constp.tile([128, 128], dt.float32)
            make_identity(nc, ident[:])
            # per-layer per-tile a_dst scores, staged in SBUF then bulk-copied
            # to the DRAM gather tables between phases
            adst_sb1 = constp.tile([128, ntiles * HEADS], dt.bfloat16)
            adst_sb2 = constp.tile([128, ntiles * HEADS], dt.bfloat16)
            adst_sb3 = constp.tile([128, ntiles], dt.bfloat16)
            # x kept SBUF-resident (in two halves) so h1 issues no
            # per-tile loads
            NTH = NTT // 2  # table tiles per half
            xhalf = constp.tile([IN_C, NTH * 128], dt.bfloat16)
            xo_res = constp.tile([IN_C, nmax], dt.bfloat16)
            nc.sync.dma_start(out=xo_res[:], in_=x_own[:])

            # ---- DRAM temporaries ----
            # loc tiles are full gather-row width so the AllGather can write
            # the wide tables directly (strided collective outs are rejected
            # by the BIR verifier, and a narrow AG + local repack costs more)
            loc12 = dram.tile([nmax, TW], dt.bfloat16)
            loc3 = dram.tile([nmax, TW3], dt.bfloat16)
            adl1 = dram.tile([nmax, TW3], dt.bfloat16)
            adl2 = dram.tile([nmax, TW3], dt.bfloat16)
            adl3 = dram.tile([nmax, TW3], dt.bfloat16)

            FCH = 13  # a_dst flush chunk, in tiles
            adfl_stage = constp.tile([128, FCH * TW3], dt.bfloat16)

            def flush_adst(adst_sb, adl, nh):
                # stage [p, t*8+h] scores into full 256B p-major rows, then
                # DMA contiguous per-partition runs (cheap descriptors)
                adlv = adl[:].rearrange("(p t) w -> p t w", t=ntiles)
                for c0 in range(0, ntiles, FCH):
                    n = min(FCH, ntiles - c0)
                    nc.vector.tensor_copy(
                        adfl_stage[:].rearrange(
                            "p (t w) -> p t w", w=TW3)[:, :n, :nh],
                        adst_sb[:, c0 * nh:(c0 + n) * nh].rearrange(
                            "p (t h) -> p t h", h=nh))
                    nc.sync.dma_start(
                        out=adlv[:, c0:c0 + n, :].rearrange(
                            "p t w -> p (t w)"),
                        in_=adfl_stage[:, :n * TW3])

            def pack12(ps, local, adst_sb, t):
                # adst_sb is the NEXT layer's table (never the one being
                # read by the current aggregate phase)
                pk = epil.tile([128, GW], dt.bfloat16, tag="pack")
                nc.scalar.copy(out=pk[:], in_=ps[:, :GW])
                nc.sync.dma_start(out=local[t * 128:(t + 1) * 128, :GW],
                                  in_=pk[:])
                nc.scalar.copy(out=adst_sb[:, t * HEADS:(t + 1) * HEADS],
                               in_=ps[:, GW:GW + HEADS])

            def pack3(ps, t):
                pk = epil.tile([128, GW3], dt.bfloat16, tag="pack")
                nc.scalar.copy(out=pk[:], in_=ps[:, :GW3])
                nc.sync.dma_start(out=loc3[t * 128:(t + 1) * 128, :GW3],
                                  in_=pk[:])
                nc.scalar.copy(out=adst_sb3[:, t:t + 1],
                               in_=ps[:, GW3:GW3 + 1])

            PKB = 7  # table tiles per batched pack write

            def h1_phase(tab1):
                # pass A: a_dst rows for own nodes
                for t in range(ntiles):
                    ps = psum_h.tile([128, PACK], dt.float32, tag="hps")
                    nc.tensor.matmul(ps[:, :HEADS],
                                     lhsT=xo_res[:, t * 128:(t + 1) * 128],
                                     rhs=rhs1d_s[:], start=True, stop=True)
                    nc.scalar.copy(out=adst_sb1[:, t * HEADS:(t + 1) * HEADS],
                                   in_=ps[:, :HEADS])
                flush_adst(adst_sb1, adl1, HEADS)
                # pass B: full [h | a_src] table, every core identically;
                # pack casts alternate ACT/DVE, table writes batched 7 tiles
                for half in range(2):
                    nc.sync.dma_start(
                        out=xhalf[:],
                        in_=xT[:, half * NTH * 128:(half + 1) * NTH * 128])
                    for G in range(NTH // PKB):
                        stage = epil.tile([128, PKB * GW], dt.bfloat16,
                                          tag="hstage")
                        for j in range(PKB):
                            Tl = G * PKB + j
                            ps = psum_h.tile([128, PACK], dt.float32,
                                             tag="hps")
                            nc.tensor.matmul(
                                ps[:, :GW],
                                lhsT=xhalf[:, Tl * 128:(Tl + 1) * 128],
                                rhs=rhs1_s[:], start=True, stop=True)
                            dstc = stage[:, j * GW:(j + 1) * GW]
                            if j % 2 == 0:
                                nc.scalar.copy(out=dstc, in_=ps[:, :GW])
                            else:
                                nc.vector.tensor_copy(dstc, ps[:, :GW])
                        r0 = (half * NTH + G * PKB) * 128
                        nc.sync.dma_start(
                            out=tab1[r0:r0 + PKB * 128, :GW].rearrange(
                                "(j p) w -> p j w", p=128),
                            in_=stage[:].rearrange("p (j w) -> p j w", w=GW))

            def allgather(local, table):
                nc.gpsimd.collective_compute(
                    "AllGather", Alu.bypass,
                    replica_groups=[list(range(NCORES))],
                    ins=[local[:].opt()], outs=[table[:].opt()])

            def epilogue12(t, ps, rhs_next_s, b_s, layer):
                # self-loops make every real node's denominator > 0; pad
                # nodes produce inf/NaN rows that are never read
                recip = epil.tile([128, HEADS], dt.float32, tag="recip")
                nc.vector.reciprocal(recip[:], ps[:, H2:H2 + HEADS])
                act = epil.tile([128, H2], dt.float32, tag="act")
                nc.vector.tensor_tensor(
                    out=act[:].rearrange("p (c h) -> p c h", h=HEADS),
                    in0=ps[:, :H2].rearrange("p (c h) -> p c h", h=HEADS),
                    in1=recip[:].unsqueeze(1).to_broadcast([128, HID, HEADS]),
                    op=Alu.mult)
                nc.vector.tensor_add(out=act[:], in0=act[:], in1=b_s[:])
                nc.scalar.activation(out=act[:], in_=act[:], func=Act.Relu)
                w = PACK if layer == 1 else OUT_C + 2
                hps = psum_h.tile([128, PACK], dt.float32, tag="hps")
                for kc in range(2):
                    tp = psum_tp.tile([128, 128], dt.float32, tag="tp")
                    nc.tensor.transpose(
                        out=tp[:], in_=act[:, kc * 128:(kc + 1) * 128],
                        identity=ident[:])
                    aT = epil.tile([128, 128], dt.bfloat16, tag="aT")
                    nc.scalar.copy(out=aT[:], in_=tp[:])
                    nc.tensor.matmul(
                        hps[:, :w], lhsT=aT[:],
                        rhs=rhs_next_s[:, kc * w:(kc + 1) * w],
                        start=(kc == 0), stop=(kc == 1))
                if layer == 1:
                    pack12(hps, loc12, adst_sb2, t)
                else:
                    pack3(hps, t)

            def epilogue3(t, ps):
                recip = epil.tile([128, 1], dt.float32, tag="recip3")
                nc.vector.reciprocal(recip[:], ps[:, OUT_C:OUT_C + 1])
                o3 = epil.tile([128, OUT_C], dt.float32, tag="o3")
                nc.vector.tensor_scalar(
                    out=o3[:], in0=ps[:, :OUT_C], scalar1=recip[:, :1],
                    scalar2=None, op0=Alu.mult)
                nc.vector.tensor_add(out=o3[:], in0=o3[:], in1=b3_s[:])
                mneg = epil.tile([128, 1], dt.float32, tag="mneg")
                nc.vector.tensor_reduce(
                    out=mneg[:], in_=o3[:], axis=mybir.AxisListType.X,
                    op=Alu.max, negate=True)
                es = epil.tile([128, OUT_C], dt.float32, tag="es")
                ssum = epil.tile([128, 1], dt.float32, tag="ssum")
                nc.scalar.activation(out=es[:], in_=o3[:], func=Act.Exp,
                                     bias=mneg[:, :1], accum_out=ssum[:, :1])
                lse = epil.tile([128, 1], dt.float32, tag="lse")
                nc.scalar.activation(out=lse[:], in_=ssum[:], func=Act.Ln)
                fin = epil.tile([128, OUT_C], dt.float32, tag="fin")
                nc.vector.tensor_scalar(
                    out=fin[:], in0=o3[:], scalar1=mneg[:, :1],
                    scalar2=lse[:, :1], op0=Alu.add, op1=Alu.subtract)
                nc.sync.dma_start(out=out[t * 128:(t + 1) * 128, :], in_=fin[:])

            def aggregate(layer, table, adl, rhs_next_s, b_s):
                if layer == 3:
                    gw, nfeat, nh, tw = GW3, OUT_C, 1, TW3
                else:
                    gw, nfeat, nh, tw = GW, H2, HEADS, TW
                KH = KSUP // 2

                def run_epilogue(t, ps):
                    if no_epil:
                        return
                    if layer == 3:
                        epilogue3(t, ps)
                    else:
                        epilogue12(t, ps, rhs_next_s, b_s, layer)

                ps_cur = None
                pending = []  # epilogues deferred one sup so their waits
                # never stall the engine queues mid-pipeline
                for sup in range(nsup):
                    for (t, ps) in pending:
                        run_epilogue(t, ps)
                    pending = []
                    idxt = sbuf.tile([128, BLK], dt.int16, tag="idxt")
                    nc.sync.dma_start(
                        out=idxt[:],
                        in_=idxs_in[:, sup * BLK:(sup + 1) * BLK])
                    sidx = idxt[:, 0:SB]
                    didx = idxt[:, SB:2 * SB]
                    segt = idxt[:, 2 * SB:BLK].bitcast(dt.float32)

                    gt = gbuf.tile([128, KSUP, tw], dt.bfloat16,
                                   tag="g3" if layer == 3 else "gt")
                    if not no_srcg:
                        for rr, (a, b, tag) in enumerate(runs_by_sup[sup]):
                            a0, b0 = a - sup * KSUP, b - sup * KSUP
                            nidx = (b - a) * 128
                            src_ap = (table[:half, :] if tag == 0
                                      else table[half:2 * half, :])
                            nc.gpsimd.dma_gather(
                                out_ap=gt[:, a0:b0, :], in_ap=src_ap,
                                idxs_ap=sidx[:, a0 * 8:b0 * 8],
                                num_idxs=nidx, num_idxs_reg=nidx,
                                elem_size=tw,
                                queue_num=(sup + rr) % 2)
                    dts = gbuf.tile([128, KSUP, TW3], dt.bfloat16, tag="dts")
                    if not no_adst:
                        nc.gpsimd.dma_gather(
                            out_ap=dts[:], in_ap=adl[:], idxs_ap=didx[:],
                            num_idxs=KSUP * 128, num_idxs_reg=KSUP * 128,
                            elem_size=TW3, queue_num=2 + sup % 2)

                    mt = None
                    if not no_mm:
                        mt = mbuf.tile([128, KSUP * 128], dt.bfloat16,
                                       tag="mt")
                        for kk in range(KSUP):
                            nc.vector.tensor_scalar(
                                out=mt[:, kk * 128:(kk + 1) * 128],
                                in0=iota_s[:],
                                scalar1=segt[:, kk:kk + 1], scalar2=None,
                                op0=Alu.is_equal)

                    if not no_vec:
                        wt = gbuf.tile([128, KSUP, nh], dt.bfloat16, tag="wt")
                        in1 = (gt[:, :, nfeat:nfeat + nh]
                               if (no_adst or no_srcg) else dts[:, :, :nh])
                        nc.vector.tensor_tensor(
                            out=wt[:], in0=gt[:, :, nfeat:nfeat + nh],
                            in1=in1, op=Alu.add)
                        nc.scalar.activation(out=wt[:], in_=wt[:],
                                             func=Act.Prelu, alpha=NEG_SLOPE)
                        nc.scalar.activation(out=wt[:], in_=wt[:],
                                             func=Act.Exp)
                        # message scaling split in chunk-halves so the first
                        # segment matmuls can start while the second half is
                        # still on DVE
                        for hh in range(2):
                            ksl = slice(hh * KH, (hh + 1) * KH)
                            if layer != 3:
                                nc.vector.tensor_tensor(
                                    out=gt[:, ksl, :nfeat].rearrange(
                                        "p k (c h) -> p k c h", h=HEADS),
                                    in0=gt[:, ksl, :nfeat].rearrange(
                                        "p k (c h) -> p k c h", h=HEADS),
                                    in1=wt[:, ksl].unsqueeze(2).to_broadcast(
                                        [128, KH, HID, HEADS]),
                                    op=Alu.mult)
                            else:
                                nc.vector.tensor_tensor(
                                    out=gt[:, ksl, :nfeat],
                                    in0=gt[:, ksl, :nfeat],
                                    in1=wt[:, ksl].to_broadcast(
                                        [128, KH, nfeat]),
                                    op=Alu.mult)
                            nc.vector.tensor_copy(
                                gt[:, ksl, nfeat:nfeat + nh], wt[:, ksl])

                    if no_mm:
                        continue
                    for kk in range(KSUP):
                        q = sup * KSUP + kk
                        t = int(tile_of_chunk[q])
                        if q == first_chunk[t]:
                            ps_cur = psum_seg.tile([128, GW], dt.float32,
                                                   tag="segps")
                        nc.tensor.matmul(
                            ps_cur[:, :gw],
                            lhsT=mt[:, kk * 128:(kk + 1) * 128],
                            rhs=gt[:, kk, :gw],
                            start=(q == first_chunk[t]),
                            stop=(q == last_chunk[t]))
                        if q == last_chunk[t]:
                            pending.append((t, ps_cur))
                for (t, ps) in pending:
                    run_epilogue(t, ps)

            for _rep in range(repeat):
                tab1 = dram.tile([NCORES * nmax, TW], dt.bfloat16,
                                 name=f"tab1_{_rep}")
                tab2 = dram.tile([NCORES * nmax, TW], dt.bfloat16,
                                 addr_space="Shared", name=f"tab2_{_rep}")
                tab3 = dram.tile([NCORES * nmax, TW3], dt.bfloat16,
                                 addr_space="Shared", name=f"tab3_{_rep}")
                h1_phase(tab1)
                if nphase >= 1:
                    aggregate(1, tab1, adl1, rhs2_s, b1_s)
                if nphase >= 2:
                    flush_adst(adst_sb2, adl2, HEADS)
                    allgather(loc12, tab2)
                    aggregate(2, tab2, adl2, rhs3_s, b2_s)
                if nphase >= 3:
                    flush_adst(adst_sb3, adl3, 1)
                    allgather(loc3, tab3)
                    aggregate(3, tab3, adl3, None, None)

    nc.compile()
    return nc


def _make_in_maps(x, g, wts):
    """Per-core input dicts. x: [N, IN_C] f32; wts: _prep_weights output."""
    rhs1, rhs1d, rhs2, rhs3, b1r, b2r, b3r = wts
    npc, nmax = g["npc"], g["nmax"]
    iota = np.tile(np.arange(128, dtype=np.float32)[None, :],
                   (128, 1)).astype(_BF16)
    xTf = np.zeros((IN_C, NCORES * nmax), _BF16)
    for k in range(NCORES):
        xTf[:, k * nmax:k * nmax + npc] = x[k * npc:(k + 1) * npc].T
    in_maps = []
    for k in range(NCORES):
        in_maps.append({
            "xT": xTf, "x_own": xTf[:, k * nmax:(k + 1) * nmax],
            "rhs1": rhs1, "rhs1d": rhs1d, "rhs2": rhs2, "rhs3": rhs3,
            "b1r": b1r, "b2r": b2r, "b3r": b3r, "iota": iota,
            "idxs": np.ascontiguousarray(g["idxc"][k]).reshape(
                128, -1),
        })
    return in_maps


_CACHE = {}


def kernel(x, edge_index, W1, as1, ad1, b1, W2, as2, ad2, b2, W3, as3, ad3, b3,
           _repeat=1):
    from concourse.bass_utils import run_bass_kernel_spmd

    x = np.asarray(x, np.float32)
    edge_index = np.asarray(edge_index)
    g = _prep_graph(edge_index)
    wts = _prep_weights(W1, as1, ad1, b1, W2, as2, ad2, b2, W3, as3, ad3, b3)

    key = (hash(edge_index.tobytes()), _repeat)
    if key not in _CACHE:
        _CACHE[key] = _build_bass(g, repeat=_repeat)
    nc = _CACHE[key]

    in_maps = _make_in_maps(x, g, wts)
    res = run_bass_kernel_spmd(nc, in_maps, core_ids=list(range(NCORES)))
    npc = g["npc"]
    outf = np.zeros((N, OUT_C), np.float32)
    for k in range(NCORES):
        outf[k * npc:(k + 1) * npc] = res.results[k]["out"][:npc]
    return outf


# revision 87
# speedup vs baseline: 1.0304x; 1.0304x over previous
"""3-layer GAT on 8 Trainium2 NeuronCores (Bass/Tile).

Edge-sharded by destination range:
  - Nodes split into 8 contiguous ranges (one per core); each core owns the
    softmax + aggregation for its destination nodes.
  - Layer 1's per-node table [h | a_src] is computed LOCALLY on every core
    from a replicated copy of x (no collective): x is an input, so each core
    can build the full 50176-row table with 392 small bf16 matmuls, which is
    much cheaper than the 38MB AllGather it replaces.  Per-core a_dst rows
    come from a second tiny matmul pass over the core's own x shard.
  - For layers 2/3 the aggregation output is only known by the dst-owning
    core, so tables are AllGathered -- but only the used columns (264 of
    384 / 48 of 128) with a strided output AP that lands the rows in the
    768B/256B-stride layout dma_gather needs.
  - Edges (with self loops) are bucketed per core into 128-dst tiles x
    128-edge chunks; chunk structure (incl. lo/hi int16-index table halves)
    is made identical across cores so one SPMD instruction stream fits all.
  - Per 8-chunk super-batch the kernel dma_gathers source rows + dest
    attention rows, computes w = exp(leaky_relu(a_src+a_dst)) (softmax
    shift-invariance removes the segment-max pass at these value ranges),
    scales messages by w, and segment-sums with matmuls against one-hot
    membership matrices, keeping numerator and denominator together in
    PSUM.  The per-tile epilogue divides, applies bias/relu, and feeds the
    next layer's matmul whose rhs [W | W@att_src | W@att_dst] also emits
    the next attention scores.
"""

import numpy as np
import ml_dtypes

N = 50000
E = 800000
IN_C = 128
HID = 32
OUT_C = 40
HEADS = 8
NEG_SLOPE = 0.2
NCORES = 8

_BF16 = ml_dtypes.bfloat16

KSUP = 8  # chunks per gather super-batch (1024-idx dma_gather limit)
SB = KSUP * 8  # int16 idx cols per sup for one index stream
BLK = 2 * SB + 2 * KSUP  # per-sup cols: sidx | didx | seg bytes


def _cmajor_perm(heads, ch):
    f_new = np.arange(heads * ch)
    return (f_new % heads) * ch + f_new // heads  # perm[new] = old


def _attn_cols(w, att):
    heads, ch = att.shape
    return np.einsum("khc,hc->kh", w.reshape(-1, heads, ch), att).astype(np.float32)


def _prep_weights(W1, as1, ad1, b1, W2, as2, ad2, b2, W3, as3, ad3, b3):
    W1 = np.asarray(W1, np.float32)
    W2 = np.asarray(W2, np.float32)
    W3 = np.asarray(W3, np.float32)
    perm = _cmajor_perm(HEADS, HID)

    rhs1 = np.concatenate(
        [W1[:, perm], _attn_cols(W1, np.asarray(as1, np.float32))],
        axis=1).astype(_BF16)
    rhs1d = _attn_cols(W1, np.asarray(ad1, np.float32)).astype(_BF16)
    W2r = W2[perm, :]
    rhs2 = np.concatenate(
        [W2r[:, perm], _attn_cols(W2r, np.asarray(as2, np.float32)),
         _attn_cols(W2r, np.asarray(ad2, np.float32))], axis=1).astype(_BF16)
    W3r = W3[perm, :]
    as3p = (W3r @ np.asarray(as3, np.float32)[0]).reshape(-1, 1)
    ad3p = (W3r @ np.asarray(ad3, np.float32)[0]).reshape(-1, 1)
    rhs3 = np.concatenate([W3r, as3p, ad3p], axis=1).astype(_BF16)

    def bcast(b):
        return np.tile(np.asarray(b, np.float32)[None, :], (128, 1))

    return (rhs1, rhs1d, rhs2, rhs3,
            bcast(np.asarray(b1, np.float32)[perm]),
            bcast(np.asarray(b2, np.float32)[perm]),
            bcast(np.asarray(b3, np.float32)))


def _prep_graph(edge_index):
    """Slot edges into the SPMD-uniform (tile, section, chunk) grid."""
    src = np.concatenate([edge_index[0], np.arange(N)]).astype(np.int64)
    dst = np.concatenate([edge_index[1], np.arange(N)]).astype(np.int64)

    npc = N // NCORES
    ntiles = (npc + 127) // 128
    nmax = ntiles * 128
    half = (NCORES // 2) * nmax

    core_of = dst // npc
    d_loc = dst - core_of * npc
    tile_of = d_loc // 128
    s_core = src // npc
    s_row = s_core * nmax + (src - s_core * npc)  # table row of src
    is_hi = s_row >= half

    cnt = np.zeros((NCORES, ntiles, 2), np.int64)
    np.add.at(cnt, (core_of, tile_of, is_hi.astype(np.int64)), 1)
    sec_cpt = np.ceil(cnt / 128).astype(np.int64).max(axis=0)  # [ntiles, 2]
    sec_cpt[:, 0] = np.maximum(sec_cpt[:, 0], 1)

    total = int(sec_cpt.sum())
    pad = (-total) % KSUP
    sec_cpt[-1, 1] += pad
    total += pad
    nsup = total // KSUP

    tile_of_chunk = []
    tag_of_chunk = []
    for t in range(ntiles):
        tile_of_chunk += [t] * int(sec_cpt[t, 0] + sec_cpt[t, 1])
        tag_of_chunk += [0] * int(sec_cpt[t, 0]) + [1] * int(sec_cpt[t, 1])
    tile_of_chunk = np.array(tile_of_chunk)
    tag_of_chunk = np.array(tag_of_chunk)
    sec_base = np.zeros((ntiles, 2), np.int64)
    sec_base.ravel()[1:] = np.cumsum(sec_cpt.ravel())[:-1]

    # combined per-sup stream: [sidx | didx | seg-bytes] int16 cols
    idxc = np.zeros((NCORES, 128, nsup, BLK), np.int16)
    seg = np.full((NCORES, nsup, 128, KSUP), 255.0, np.float32)

    order = np.lexsort((src, is_hi, tile_of, core_of))
    src_o = s_row[order]
    dst_o = d_loc[order]
    core_o = core_of[order]
    tile_o = tile_of[order]
    hi_o = is_hi[order]

    for k in range(NCORES):
        m = core_o == k
        t = tile_o[m]
        hi = hi_o[m].astype(np.int64)
        sr = src_o[m] - hi * half
        dl = dst_o[m]
        key = t * 2 + hi
        cnts = np.bincount(key, minlength=ntiles * 2)
        st = np.zeros(ntiles * 2, np.int64)
        st[1:] = np.cumsum(cnts)[:-1]
        pos = np.arange(len(t)) - st[key]
        q = sec_base[t, hi] + pos // 128
        p = pos % 128
        colsup = q // KSUP
        col = (q % KSUP) * 8 + p // 16
        row = p % 16
        # a_dst table rows are p-major (row = (d%128)*ntiles + d//128) so
        # the SBUF->DRAM flush is 128 contiguous runs instead of 6k tiny ones
        dlp = (dl % 128) * ntiles + dl // 128
        for c in range(8):
            idxc[k, row + 16 * c, colsup, col] = sr
            idxc[k, row + 16 * c, colsup, SB + col] = dlp
        seg[k, q // KSUP, p, q % KSUP] = (dl % 128).astype(np.float32)
    for k in range(NCORES):
        idxc[k, :, :, 2 * SB:] = np.ascontiguousarray(
            seg[k].transpose(1, 0, 2)).view(np.int16).reshape(
            128, nsup, 2 * KSUP)

    runs = []  # (sup, chunk_lo, chunk_hi, tag)
    for s in range(nsup):
        q0 = s * KSUP
        r0 = q0
        for q in range(q0 + 1, q0 + KSUP + 1):
            if q == q0 + KSUP or tag_of_chunk[q] != tag_of_chunk[r0]:
                runs.append((s, r0, q, int(tag_of_chunk[r0])))
                r0 = q

    return dict(
        idxc=idxc,
        tile_of_chunk=tile_of_chunk, runs=runs, nsup=nsup, total=total,
        ntiles=ntiles, nmax=nmax, npc=npc, half=half,
    )


def _build_bass(g, repeat=1):
    import os
    import concourse.bacc as bacc
    import concourse.mybir as mybir
    import concourse.tile as tile
    from concourse.masks import make_identity

    dt = mybir.dt
    Alu = mybir.AluOpType
    Act = mybir.ActivationFunctionType

    ntiles, nmax, nsup, total = g["ntiles"], g["nmax"], g["nsup"], g["total"]
    half = g["half"]
    tile_of_chunk = g["tile_of_chunk"]
    H2 = HEADS * HID  # 256
    GW = H2 + HEADS  # 264 useful table cols: h + a_src
    PACK = GW + HEADS  # 272 psum width in epilogue: h + a_src + a_dst
    TW = 384  # gather-table row width (768B)
    TW3 = 128  # layer-3 / a_dst table row width (256B)
    GW3 = OUT_C + 1  # 41
    NTT = NCORES * ntiles  # total table tiles

    first_chunk = {}
    last_chunk = {}
    for q, t in enumerate(tile_of_chunk):
        first_chunk.setdefault(int(t), q)
        last_chunk[int(t)] = q
    runs_by_sup = {}
    for (s, a, b, tag) in g["runs"]:
        runs_by_sup.setdefault(s, []).append((a, b, tag))

    nphase = int(os.environ.get("GAT_PHASES", "3"))
    # timing-only knock-outs (break correctness; for bottleneck isolation)
    no_srcg = int(os.environ.get("GAT_NO_SRCG", "0"))
    no_adst = int(os.environ.get("GAT_NO_ADST", "0"))
    no_vec = int(os.environ.get("GAT_NO_VEC", "0"))
    no_mm = int(os.environ.get("GAT_NO_MM", "0"))
    no_epil = int(os.environ.get("GAT_NO_EPIL", "0"))
    gbufs = int(os.environ.get("GAT_GBUFS", "8"))

    nc = bacc.Bacc("TRN2", target_bir_lowering=False, debug=False,
                   num_devices=NCORES, num_swdge_queues=4)

    xT = nc.dram_tensor("xT", [IN_C, NCORES * nmax], dt.bfloat16,
                        kind="ExternalInput")
    x_own = nc.dram_tensor("x_own", [IN_C, nmax], dt.bfloat16,
                           kind="ExternalInput")
    rhs1 = nc.dram_tensor("rhs1", [IN_C, GW], dt.bfloat16,
                          kind="ExternalInput")
    rhs1d = nc.dram_tensor("rhs1d", [IN_C, HEADS], dt.bfloat16,
                           kind="ExternalInput")
    rhs2 = nc.dram_tensor("rhs2", [H2, PACK], dt.bfloat16,
                          kind="ExternalInput")
    rhs3 = nc.dram_tensor("rhs3", [H2, OUT_C + 2], dt.bfloat16,
                          kind="ExternalInput")
    b1r = nc.dram_tensor("b1r", [128, H2], dt.float32, kind="ExternalInput")
    b2r = nc.dram_tensor("b2r", [128, H2], dt.float32, kind="ExternalInput")
    b3r = nc.dram_tensor("b3r", [128, OUT_C], dt.float32, kind="ExternalInput")
    iota = nc.dram_tensor("iota", [128, 128], dt.bfloat16, kind="ExternalInput")
    idxs_in = nc.dram_tensor("idxs", [128, nsup * BLK], dt.int16,
                             kind="ExternalInput")
    out = nc.dram_tensor("out", [nmax, OUT_C], dt.float32,
                         kind="ExternalOutput")

    with tile.TileContext(nc) as tc:
        with (
            tc.tile_pool(name="const", bufs=1) as constp,
            tc.tile_pool(name="sbuf", bufs=6) as sbuf,
            tc.tile_pool(name="gbuf", bufs=gbufs) as gbuf,
            tc.tile_pool(name="mbuf", bufs=6) as mbuf,
            tc.tile_pool(name="epil", bufs=2) as epil,
            tc.tile_pool(name="psum_seg", bufs=4, space="PSUM") as psum_seg,
            tc.tile_pool(name="psum_h", bufs=2, space="PSUM") as psum_h,
            tc.tile_pool(name="psum_tp", bufs=2, space="PSUM") as psum_tp,
            tc.tile_pool(name="dram", bufs=1, space="DRAM") as dram,
        ):
            # ---- constants ----
            rhs1_s = constp.tile([IN_C, GW], dt.bfloat16)
            nc.sync.dma_start(out=rhs1_s[:], in_=rhs1[:])
            rhs1d_s = constp.tile([IN_C, HEADS], dt.bfloat16)
            nc.sync.dma_start(out=rhs1d_s[:], in_=rhs1d[:])
            rhs2_s = constp.tile([128, 2 * PACK], dt.bfloat16)
            nc.sync.dma_start(
                out=rhs2_s[:].rearrange("p (k f) -> p k f", k=2),
                in_=rhs2[:].rearrange("(k p) f -> p k f", p=128))
            rhs3_s = constp.tile([128, 2 * (OUT_C + 2)], dt.bfloat16)
            nc.sync.dma_start(
                out=rhs3_s[:].rearrange("p (k f) -> p k f", k=2),
                in_=rhs3[:].rearrange("(k p) f -> p k f", p=128))
            b1_s = constp.tile([128, H2], dt.float32)
            nc.sync.dma_start(out=b1_s[:], in_=b1r[:])
            b2_s = constp.tile([128, H2], dt.float32)
            nc.sync.dma_start(out=b2_s[:], in_=b2r[:])
            b3_s = constp.tile([128, OUT_C], dt.float32)
            nc.sync.dma_start(out=b3_s[:], in_=b3r[:])
            iota_s = constp.tile([128, 128], dt.bfloat16)
            nc.sync.dma_start(out=iota_s[:], in_=iota[:])
            ident = constp.tile([128, 128], dt.float32)
            make_identity(nc, ident[:])
            # per-layer per-tile a_dst scores, staged in SBUF then bulk-copied
            # to the DRAM gather tables between phases
            adst_sb1 = constp.tile([128, ntiles * HEADS], dt.bfloat16)
            adst_sb2 = constp.tile([128, ntiles * HEADS], dt.bfloat16)
            adst_sb3 = constp.tile([128, ntiles], dt.bfloat16)
            # x kept SBUF-resident (in two halves) so h1 issues no
            # per-tile loads
            NTH = NTT // 2  # table tiles per half
            xhalf = constp.tile([IN_C, NTH * 128], dt.bfloat16)
            xo_res = constp.tile([IN_C, nmax], dt.bfloat16)
            nc.sync.dma_start(out=xo_res[:], in_=x_own[:])

            # ---- DRAM temporaries ----
            # loc tiles are full gather-row width so the AllGather can write
            # the wide tables directly (strided collective outs are rejected
            # by the BIR verifier, and a narrow AG + local repack costs more)
            loc12 = dram.tile([nmax, TW], dt.bfloat16)
            loc3 = dram.tile([nmax, TW3], dt.bfloat16)
            adl1 = dram.tile([nmax, TW3], dt.bfloat16)
            adl2 = dram.tile([nmax, TW3], dt.bfloat16)
            adl3 = dram.tile([nmax, TW3], dt.bfloat16)

            FCH = 13  # a_dst flush chunk, in tiles
            adfl_stage = constp.tile([128, FCH * TW3], dt.bfloat16)

            def flush_adst(adst_sb, adl, nh):
                # stage [p, t*8+h] scores into full 256B p-major rows, then
                # DMA contiguous per-partition runs (cheap descriptors)
                adlv = adl[:].rearrange("(p t) w -> p t w", t=ntiles)
                for c0 in range(0, ntiles, FCH):
                    n = min(FCH, ntiles - c0)
                    nc.vector.tensor_copy(
                        adfl_stage[:].rearrange(
                            "p (t w) -> p t w", w=TW3)[:, :n, :nh],
                        adst_sb[:, c0 * nh:(c0 + n) * nh].rearrange(
                            "p (t h) -> p t h", h=nh))
                    nc.sync.dma_start(
                        out=adlv[:, c0:c0 + n, :].rearrange(
                            "p t w -> p (t w)"),
                        in_=adfl_stage[:, :n * TW3])

            def pack12(ps, local, adst_sb, t):
                # adst_sb is the NEXT layer's table (never the one being
                # read by the current aggregate phase)
                pk = epil.tile([128, GW], dt.bfloat16, tag="pack")
                nc.scalar.copy(out=pk[:], in_=ps[:, :GW])
                nc.sync.dma_start(out=local[t * 128:(t + 1) * 128, :GW],
                                  in_=pk[:])
                nc.scalar.copy(out=adst_sb[:, t * HEADS:(t + 1) * HEADS],
                               in_=ps[:, GW:GW + HEADS])

            def pack3(ps, t):
                pk = epil.tile([128, GW3], dt.bfloat16, tag="pack")
                nc.scalar.copy(out=pk[:], in_=ps[:, :GW3])
                nc.sync.dma_start(out=loc3[t * 128:(t + 1) * 128, :GW3],
                                  in_=pk[:])
                nc.scalar.copy(out=adst_sb3[:, t:t + 1],
                               in_=ps[:, GW3:GW3 + 1])

            PKB = 7  # table tiles per batched pack write

            def h1_phase(tab1):
                # pass A: a_dst rows for own nodes
                for t in range(ntiles):
                    ps = psum_h.tile([128, PACK], dt.float32, tag="hps")
                    nc.tensor.matmul(ps[:, :HEADS],
                                     lhsT=xo_res[:, t * 128:(t + 1) * 128],
                                     rhs=rhs1d_s[:], start=True, stop=True)
                    nc.scalar.copy(out=adst_sb1[:, t * HEADS:(t + 1) * HEADS],
                                   in_=ps[:, :HEADS])
                flush_adst(adst_sb1, adl1, HEADS)
                # pass B: full [h | a_src] table, every core identically;
                # pack casts alternate ACT/DVE, table writes batched 7 tiles
                for half in range(2):
                    nc.sync.dma_start(
                        out=xhalf[:],
                        in_=xT[:, half * NTH * 128:(half + 1) * NTH * 128])
                    for G in range(NTH // PKB):
                        stage = epil.tile([128, PKB * GW], dt.bfloat16,
                                          tag="hstage")
                        for j in range(PKB):
                            Tl = G * PKB + j
                            ps = psum_h.tile([128, PACK], dt.float32,
                                             tag="hps")
                            nc.tensor.matmul(
                                ps[:, :GW],
                                lhsT=xhalf[:, Tl * 128:(Tl + 1) * 128],
                                rhs=rhs1_s[:], start=True, stop=True)
                            dstc = stage[:, j * GW:(j + 1) * GW]
                            if j % 2 == 0:
                                nc.scalar.copy(out=dstc, in_=ps[:, :GW])
                            else:
                                nc.vector.tensor_copy(dstc, ps[:, :GW])
                        r0 = (half * NTH + G * PKB) * 128
                        nc.sync.dma_start(
                            out=tab1[r0:r0 + PKB * 128, :GW].rearrange(
                                "(j p) w -> p j w", p=128),
                            in_=stage[:].rearrange("p (j w) -> p j w", w=GW))

            def allgather(local, table):
                nc.gpsimd.collective_compute(
                    "AllGather", Alu.bypass,
                    replica_groups=[list(range(NCORES))],
                    ins=[local[:].opt()], outs=[table[:].opt()])

            def epilogue12(t, ps, rhs_next_s, b_s, layer):
                # self-loops make every real node's denominator > 0; pad
                # nodes produce inf/NaN rows that are never read
                recip = epil.tile([128, HEADS], dt.float32, tag="recip")
                nc.vector.reciprocal(recip[:], ps[:, H2:H2 + HEADS])
                act = epil.tile([128, H2], dt.float32, tag="act")
                nc.vector.tensor_tensor(
                    out=act[:].rearrange("p (c h) -> p c h", h=HEADS),
                    in0=ps[:, :H2].rearrange("p (c h) -> p c h", h=HEADS),
                    in1=recip[:].unsqueeze(1).to_broadcast([128, HID, HEADS]),
                    op=Alu.mult)
                nc.vector.tensor_add(out=act[:], in0=act[:], in1=b_s[:])
                nc.scalar.activation(out=act[:], in_=act[:], func=Act.Relu)
                w = PACK if layer == 1 else OUT_C + 2
                hps = psum_h.tile([128, PACK], dt.float32, tag="hps")
                for kc in range(2):
                    tp = psum_tp.tile([128, 128], dt.float32, tag="tp")
                    nc.tensor.transpose(
                        out=tp[:], in_=act[:, kc * 128:(kc + 1) * 128],
                        identity=ident[:])
                    aT = epil.tile([128, 128], dt.bfloat16, tag="aT")
                    nc.scalar.copy(out=aT[:], in_=tp[:])
                    nc.tensor.matmul(
                        hps[:, :w], lhsT=aT[:],
                        rhs=rhs_next_s[:, kc * w:(kc + 1) * w],
                        start=(kc == 0), stop=(kc == 1))
                if layer == 1:
                    pack12(hps, loc12, adst_sb2, t)
                else:
                    pack3(hps, t)

            def epilogue3(t, ps):
                recip = epil.tile([128, 1], dt.float32, tag="recip3")
                nc.vector.reciprocal(recip[:], ps[:, OUT_C:OUT_C + 1])
                o3 = epil.tile([128, OUT_C], dt.float32, tag="o3")
                nc.vector.tensor_scalar(
                    out=o3[:], in0=ps[:, :OUT_C], scalar1=recip[:, :1],
                    scalar2=None, op0=Alu.mult)
                nc.vector.tensor_add(out=o3[:], in0=o3[:], in1=b3_s[:])
                mneg = epil.tile([128, 1], dt.float32, tag="mneg")
                nc.vector.tensor_reduce(
                    out=mneg[:], in_=o3[:], axis=mybir.AxisListType.X,
                    op=Alu.max, negate=True)
                es = epil.tile([128, OUT_C], dt.float32, tag="es")
                ssum = epil.tile([128, 1], dt.float32, tag="ssum")
                nc.scalar.activation(out=es[:], in_=o3[:], func=Act.Exp,
                                     bias=mneg[:, :1], accum_out=ssum[:, :1])
                lse = epil.tile([128, 1], dt.float32, tag="lse")
                nc.scalar.activation(out=lse[:], in_=ssum[:], func=Act.Ln)
                fin = epil.tile([128, OUT_C], dt.float32, tag="fin")
                nc.vector.tensor_scalar(
                    out=fin[:], in0=o3[:], scalar1=mneg[:, :1],
                    scalar2=lse[:, :1], op0=Alu.add, op1=Alu.subtract)
                nc.sync.dma_start(out=out[t * 128:(t + 1) * 128, :], in_=fin[:])

            def aggregate(layer, table, adl, rhs_next_s, b_s):
                if layer == 3:
                    gw, nfeat, nh, tw = GW3, OUT_C, 1, TW3
                else:
                    gw, nfeat, nh, tw = GW, H2, HEADS, TW
                KH = KSUP // 2

                def run_epilogue(t, ps):
                    if no_epil:
                        return
                    if layer == 3:
                        epilogue3(t, ps)
                    else:
                        epilogue12(t, ps, rhs_next_s, b_s, layer)

                ps_cur = None
                pending = []  # epilogues deferred one sup so their waits
                # never stall the engine queues mid-pipeline
                for sup in range(nsup):
                    for (t, ps) in pending:
                        run_epilogue(t, ps)
                    pending = []
                    idxt = sbuf.tile([128, BLK], dt.int16, tag="idxt")
                    nc.sync.dma_start(
                        out=idxt[:],
                        in_=idxs_in[:, sup * BLK:(sup + 1) * BLK])
                    sidx = idxt[:, 0:SB]
                    didx = idxt[:, SB:2 * SB]
                    segt = idxt[:, 2 * SB:BLK].bitcast(dt.float32)

                    gt = gbuf.tile([128, KSUP, tw], dt.bfloat16,
                                   tag="g3" if layer == 3 else "gt")
                    if not no_srcg:
                        for rr, (a, b, tag) in enumerate(runs_by_sup[sup]):
                            a0, b0 = a - sup * KSUP, b - sup * KSUP
                            nidx = (b - a) * 128
                            src_ap = (table[:half, :] if tag == 0
                                      else table[half:2 * half, :])
                            nc.gpsimd.dma_gather(
                                out_ap=gt[:, a0:b0, :], in_ap=src_ap,
                                idxs_ap=sidx[:, a0 * 8:b0 * 8],
                                num_idxs=nidx, num_idxs_reg=nidx,
                                elem_size=tw,
                                queue_num=(sup + rr) % 2)
                    dts = gbuf.tile([128, KSUP, TW3], dt.bfloat16, tag="dts")
                    if not no_adst:
                        nc.gpsimd.dma_gather(
                            out_ap=dts[:], in_ap=adl[:], idxs_ap=didx[:],
                            num_idxs=KSUP * 128, num_idxs_reg=KSUP * 128,
                            elem_size=TW3, queue_num=2 + sup % 2)

                    mt = None
                    if not no_mm:
                        mt = mbuf.tile([128, KSUP * 128], dt.bfloat16,
                                       tag="mt")
                        for kk in range(KSUP):
                            nc.vector.tensor_scalar(
                                out=mt[:, kk * 128:(kk + 1) * 128],
                                in0=iota_s[:],
                                scalar1=segt[:, kk:kk + 1], scalar2=None,
                                op0=Alu.is_equal)

                    if not no_vec:
                        wt = gbuf.tile([128, KSUP, nh], dt.bfloat16, tag="wt")
                        in1 = (gt[:, :, nfeat:nfeat + nh]
                               if (no_adst or no_srcg) else dts[:, :, :nh])
                        nc.vector.tensor_tensor(
                            out=wt[:], in0=gt[:, :, nfeat:nfeat + nh],
                            in1=in1, op=Alu.add)
                        nc.scalar.activation(out=wt[:], in_=wt[:],
                                             func=Act.Prelu, alpha=NEG_SLOPE)
                        nc.scalar.activation(out=wt[:], in_=wt[:],
                                             func=Act.Exp)
                        # message scaling split in chunk-halves so the first
                        # segment matmuls can start while the second half is
                        # still on DVE
                        for hh in range(2):
                            ksl = slice(hh * KH, (hh + 1) * KH)
                            if layer != 3:
                                nc.vector.tensor_tensor(
                                    out=gt[:, ksl, :nfeat].rearrange(
                                        "p k (c h) -> p k c h", h=HEADS),
                                    in0=gt[:, ksl, :nfeat].rearrange(
                                        "p k (c h) -> p k c h", h=HEADS),
                                    in1=wt[:, ksl].unsqueeze(2).to_broadcast(
                                        [128, KH, HID, HEADS]),
                                    op=Alu.mult)
                            else:
                                nc.vector.tensor_tensor(
                                    out=gt[:, ksl, :nfeat],
                                    in0=gt[:, ksl, :nfeat],
                                    in1=wt[:, ksl].to_broadcast(
                                        [128, KH, nfeat]),
                                    op=Alu.mult)
                            nc.vector.tensor_copy(
                                gt[:, ksl, nfeat:nfeat + nh], wt[:, ksl])

                    if no_mm:
                        continue
                    for kk in range(KSUP):
                        q = sup * KSUP + kk
                        t = int(tile_of_chunk[q])
                        if q == first_chunk[t]:
                            ps_cur = psum_seg.tile([128, GW], dt.float32,
                                                   tag="segps")
                        nc.tensor.matmul(
                            ps_cur[:, :gw],
                            lhsT=mt[:, kk * 128:(kk + 1) * 128],
                            rhs=gt[:, kk, :gw],
                            start=(q == first_chunk[t]),
                            stop=(q == last_chunk[t]))
                        if q == last_chunk[t]:
                            pending.append((t, ps_cur))
                for (t, ps) in pending:
                    run_epilogue(t, ps)

            for _rep in range(repeat):
                tab1 = dram.tile([NCORES * nmax, TW], dt.bfloat16,
                                 name=f"tab1_{_rep}")
                tab2 = dram.tile([NCORES * nmax, TW], dt.bfloat16,
                                 addr_space="Shared", name=f"tab2_{_rep}")
                tab3 = dram.tile([NCORES * nmax, TW3], dt.bfloat16,
                                 addr_space="Shared", name=f"tab3_{_rep}")
                h1_phase(tab1)
                if nphase >= 1:
                    aggregate(1, tab1, adl1, rhs2_s, b1_s)
                if nphase >= 2:
                    flush_adst(adst_sb2, adl2, HEADS)
                    allgather(loc12, tab2)
                    aggregate(2, tab2, adl2, rhs3_s, b2_s)
                if nphase >= 3:
                    flush_adst(adst_sb3, adl3, 1)
                    allgather(loc3, tab3)
                    aggregate(3, tab3, adl3, None, None)

    nc.compile()
    return nc


def _make_in_maps(x, g, wts):
    """Per-core input dicts. x: [N, IN_C] f32; wts: _prep_weights output."""
    rhs1, rhs1d, rhs2, rhs3, b1r, b2r, b3r = wts
    npc, nmax = g["npc"], g["nmax"]
    iota = np.tile(np.arange(128, dtype=np.float32)[None, :],
                   (128, 1)).astype(_BF16)
    xTf = np.zeros((IN_C, NCORES * nmax), _BF16)
    for k in range(NCORES):
        xTf[:, k * nmax:k * nmax + npc] = x[k * npc:(k + 1) * npc].T
    in_maps = []
    for k in range(NCORES):
        in_maps.append({
            "xT": xTf, "x_own": xTf[:, k * nmax:(k + 1) * nmax],
            "rhs1": rhs1, "rhs1d": rhs1d, "rhs2": rhs2, "rhs3": rhs3,
            "b1r": b1r, "b2r": b2r, "b3r": b3r, "iota": iota,
            "idxs": np.ascontiguousarray(g["idxc"][k]).reshape(
                128, -1),
        })
    return in_maps


_CACHE = {}


def kernel(x, edge_index, W1, as1, ad1, b1, W2, as2, ad2, b2, W3, as3, ad3, b3,
           _repeat=1):
    from concourse.bass_utils import run_bass_kernel_spmd

    x = np.asarray(x, np.float32)
    edge_index = np.asarray(edge_index)
    g = _prep_graph(edge_index)
    wts = _prep_weights(W1, as1, ad1, b1, W2, as2, ad2, b2, W3, as3, ad3, b3)

    key = (hash(edge_index.tobytes()), _repeat)
    if key not in _CACHE:
        _CACHE[key] = _build_bass(g, repeat=_repeat)
    nc = _CACHE[key]

    in_maps = _make_in_maps(x, g, wts)
    res = run_bass_kernel_spmd(nc, in_maps, core_ids=list(range(NCORES)))
    npc = g["npc"]
    outf = np.zeros((N, OUT_C), np.float32)
    for k in range(NCORES):
        outf[k * npc:(k + 1) * npc] = res.results[k]["out"][:npc]
    return outf


# revision 88
# speedup vs baseline: 1.0719x; 1.0403x over previous
"""3-layer GAT on 8 Trainium2 NeuronCores (Bass/Tile).

Edge-sharded by destination range:
  - Nodes split into 8 contiguous ranges (one per core); each core owns the
    softmax + aggregation for its destination nodes.
  - Layer 1's per-node table [h | a_src] is computed LOCALLY on every core
    from a replicated copy of x (no collective): x is an input, so each core
    can build the full 50176-row table with 392 small bf16 matmuls, which is
    much cheaper than the 38MB AllGather it replaces.  Per-core a_dst rows
    come from a second tiny matmul pass over the core's own x shard.
  - For layers 2/3 the aggregation output is only known by the dst-owning
    core, so tables are AllGathered -- but only the used columns (264 of
    384 / 48 of 128) with a strided output AP that lands the rows in the
    768B/256B-stride layout dma_gather needs.
  - Edges (with self loops) are bucketed per core into 128-dst tiles x
    128-edge chunks; chunk structure (incl. lo/hi int16-index table halves)
    is made identical across cores so one SPMD instruction stream fits all.
  - Per 8-chunk super-batch the kernel dma_gathers source rows + dest
    attention rows, computes w = exp(leaky_relu(a_src+a_dst)) (softmax
    shift-invariance removes the segment-max pass at these value ranges),
    scales messages by w, and segment-sums with matmuls against one-hot
    membership matrices, keeping numerator and denominator together in
    PSUM.  The per-tile epilogue divides, applies bias/relu, and feeds the
    next layer's matmul whose rhs [W | W@att_src | W@att_dst] also emits
    the next attention scores.
"""

import numpy as np
import ml_dtypes

N = 50000
E = 800000
IN_C = 128
HID = 32
OUT_C = 40
HEADS = 8
NEG_SLOPE = 0.2
NCORES = 8

_BF16 = ml_dtypes.bfloat16

KSUP = 8  # chunks per gather super-batch (1024-idx dma_gather limit)
SB = KSUP * 8  # int16 idx cols per sup for one index stream
BLK = 2 * SB + 2 * KSUP  # per-sup cols: sidx | didx | seg bytes


def _cmajor_perm(heads, ch):
    f_new = np.arange(heads * ch)
    return (f_new % heads) * ch + f_new // heads  # perm[new] = old


def _attn_cols(w, att):
    heads, ch = att.shape
    return np.einsum("khc,hc->kh", w.reshape(-1, heads, ch), att).astype(np.float32)


def _prep_weights(W1, as1, ad1, b1, W2, as2, ad2, b2, W3, as3, ad3, b3):
    W1 = np.asarray(W1, np.float32)
    W2 = np.asarray(W2, np.float32)
    W3 = np.asarray(W3, np.float32)
    perm = _cmajor_perm(HEADS, HID)

    rhs1 = np.concatenate(
        [W1[:, perm], _attn_cols(W1, np.asarray(as1, np.float32))],
        axis=1).astype(_BF16)
    rhs1d = _attn_cols(W1, np.asarray(ad1, np.float32)).astype(_BF16)
    W2r = W2[perm, :]
    rhs2 = np.concatenate(
        [W2r[:, perm], _attn_cols(W2r, np.asarray(as2, np.float32)),
         _attn_cols(W2r, np.asarray(ad2, np.float32))], axis=1).astype(_BF16)
    W3r = W3[perm, :]
    as3p = (W3r @ np.asarray(as3, np.float32)[0]).reshape(-1, 1)
    ad3p = (W3r @ np.asarray(ad3, np.float32)[0]).reshape(-1, 1)
    rhs3 = np.concatenate([W3r, as3p, ad3p], axis=1).astype(_BF16)

    def bcast(b):
        return np.tile(np.asarray(b, np.float32)[None, :], (128, 1))

    return (rhs1, rhs1d, rhs2, rhs3,
            bcast(np.asarray(b1, np.float32)[perm]),
            bcast(np.asarray(b2, np.float32)[perm]),
            bcast(np.asarray(b3, np.float32)))


def _prep_graph(edge_index):
    """Slot edges into the SPMD-uniform (tile, section, chunk) grid."""
    src = np.concatenate([edge_index[0], np.arange(N)]).astype(np.int64)
    dst = np.concatenate([edge_index[1], np.arange(N)]).astype(np.int64)

    npc = N // NCORES
    ntiles = (npc + 127) // 128
    nmax = ntiles * 128
    half = (NCORES // 2) * nmax

    core_of = dst // npc
    d_loc = dst - core_of * npc
    tile_of = d_loc // 128
    s_core = src // npc
    s_row = s_core * nmax + (src - s_core * npc)  # table row of src
    is_hi = s_row >= half

    cnt = np.zeros((NCORES, ntiles, 2), np.int64)
    np.add.at(cnt, (core_of, tile_of, is_hi.astype(np.int64)), 1)
    sec_cpt = np.ceil(cnt / 128).astype(np.int64).max(axis=0)  # [ntiles, 2]
    sec_cpt[:, 0] = np.maximum(sec_cpt[:, 0], 1)

    total = int(sec_cpt.sum())
    pad = (-total) % KSUP
    sec_cpt[-1, 1] += pad
    total += pad
    nsup = total // KSUP

    tile_of_chunk = []
    tag_of_chunk = []
    for t in range(ntiles):
        tile_of_chunk += [t] * int(sec_cpt[t, 0] + sec_cpt[t, 1])
        tag_of_chunk += [0] * int(sec_cpt[t, 0]) + [1] * int(sec_cpt[t, 1])
    tile_of_chunk = np.array(tile_of_chunk)
    tag_of_chunk = np.array(tag_of_chunk)
    sec_base = np.zeros((ntiles, 2), np.int64)
    sec_base.ravel()[1:] = np.cumsum(sec_cpt.ravel())[:-1]

    # combined per-sup stream: [sidx | didx | seg-bytes] int16 cols
    idxc = np.zeros((NCORES, 128, nsup, BLK), np.int16)
    seg = np.full((NCORES, nsup, 128, KSUP), 255.0, np.float32)

    order = np.lexsort((src, is_hi, tile_of, core_of))
    src_o = s_row[order]
    dst_o = d_loc[order]
    core_o = core_of[order]
    tile_o = tile_of[order]
    hi_o = is_hi[order]

    for k in range(NCORES):
        m = core_o == k
        t = tile_o[m]
        hi = hi_o[m].astype(np.int64)
        sr = src_o[m] - hi * half
        dl = dst_o[m]
        key = t * 2 + hi
        cnts = np.bincount(key, minlength=ntiles * 2)
        st = np.zeros(ntiles * 2, np.int64)
        st[1:] = np.cumsum(cnts)[:-1]
        pos = np.arange(len(t)) - st[key]
        q = sec_base[t, hi] + pos // 128
        p = pos % 128
        colsup = q // KSUP
        col = (q % KSUP) * 8 + p // 16
        row = p % 16
        # a_dst table rows are p-major (row = (d%128)*ntiles + d//128) so
        # the SBUF->DRAM flush is 128 contiguous runs instead of 6k tiny ones
        dlp = (dl % 128) * ntiles + dl // 128
        for c in range(8):
            idxc[k, row + 16 * c, colsup, col] = sr
            idxc[k, row + 16 * c, colsup, SB + col] = dlp
        seg[k, q // KSUP, p, q % KSUP] = (dl % 128).astype(np.float32)
    for k in range(NCORES):
        idxc[k, :, :, 2 * SB:] = np.ascontiguousarray(
            seg[k].transpose(1, 0, 2)).view(np.int16).reshape(
            128, nsup, 2 * KSUP)

    runs = []  # (sup, chunk_lo, chunk_hi, tag)
    for s in range(nsup):
        q0 = s * KSUP
        r0 = q0
        for q in range(q0 + 1, q0 + KSUP + 1):
            if q == q0 + KSUP or tag_of_chunk[q] != tag_of_chunk[r0]:
                runs.append((s, r0, q, int(tag_of_chunk[r0])))
                r0 = q

    return dict(
        idxc=idxc,
        tile_of_chunk=tile_of_chunk, runs=runs, nsup=nsup, total=total,
        ntiles=ntiles, nmax=nmax, npc=npc, half=half,
    )


def _build_bass(g, repeat=1):
    import os
    import concourse.bacc as bacc
    import concourse.mybir as mybir
    import concourse.tile as tile
    from concourse.masks import make_identity

    dt = mybir.dt
    Alu = mybir.AluOpType
    Act = mybir.ActivationFunctionType

    ntiles, nmax, nsup, total = g["ntiles"], g["nmax"], g["nsup"], g["total"]
    half = g["half"]
    tile_of_chunk = g["tile_of_chunk"]
    H2 = HEADS * HID  # 256
    GW = H2 + HEADS  # 264 useful table cols: h + a_src
    PACK = GW + HEADS  # 272 psum width in epilogue: h + a_src + a_dst
    TW = 384  # gather-table row width (768B)
    TW3 = 128  # layer-3 / a_dst table row width (256B)
    GW3 = OUT_C + 1  # 41
    NTT = NCORES * ntiles  # total table tiles

    first_chunk = {}
    last_chunk = {}
    for q, t in enumerate(tile_of_chunk):
        first_chunk.setdefault(int(t), q)
        last_chunk[int(t)] = q
    runs_by_sup = {}
    for (s, a, b, tag) in g["runs"]:
        runs_by_sup.setdefault(s, []).append((a, b, tag))

    nphase = int(os.environ.get("GAT_PHASES", "3"))
    # timing-only knock-outs (break correctness; for bottleneck isolation)
    no_srcg = int(os.environ.get("GAT_NO_SRCG", "0"))
    no_adst = int(os.environ.get("GAT_NO_ADST", "0"))
    no_vec = int(os.environ.get("GAT_NO_VEC", "0"))
    no_mm = int(os.environ.get("GAT_NO_MM", "0"))
    no_epil = int(os.environ.get("GAT_NO_EPIL", "0"))
    gbufs = int(os.environ.get("GAT_GBUFS", "8"))

    nc = bacc.Bacc("TRN2", target_bir_lowering=False, debug=False,
                   num_devices=NCORES, num_swdge_queues=4)

    xT = nc.dram_tensor("xT", [IN_C, NCORES * nmax], dt.bfloat16,
                        kind="ExternalInput")
    x_own = nc.dram_tensor("x_own", [IN_C, nmax], dt.bfloat16,
                           kind="ExternalInput")
    rhs1 = nc.dram_tensor("rhs1", [IN_C, GW], dt.bfloat16,
                          kind="ExternalInput")
    rhs1d = nc.dram_tensor("rhs1d", [IN_C, HEADS], dt.bfloat16,
                           kind="ExternalInput")
    rhs2 = nc.dram_tensor("rhs2", [H2, PACK], dt.bfloat16,
                          kind="ExternalInput")
    rhs3 = nc.dram_tensor("rhs3", [H2, OUT_C + 2], dt.bfloat16,
                          kind="ExternalInput")
    b1r = nc.dram_tensor("b1r", [128, H2], dt.float32, kind="ExternalInput")
    b2r = nc.dram_tensor("b2r", [128, H2], dt.float32, kind="ExternalInput")
    b3r = nc.dram_tensor("b3r", [128, OUT_C], dt.float32, kind="ExternalInput")
    iota = nc.dram_tensor("iota", [128, 128], dt.bfloat16, kind="ExternalInput")
    idxs_in = nc.dram_tensor("idxs", [128, nsup * BLK], dt.int16,
                             kind="ExternalInput")
    out = nc.dram_tensor("out", [nmax, OUT_C], dt.float32,
                         kind="ExternalOutput")

    with tile.TileContext(nc) as tc:
        with (
            tc.tile_pool(name="const", bufs=1) as constp,
            tc.tile_pool(name="sbuf", bufs=6) as sbuf,
            tc.tile_pool(name="gbuf", bufs=gbufs) as gbuf,
            tc.tile_pool(name="mbuf", bufs=6) as mbuf,
            tc.tile_pool(name="epil", bufs=2) as epil,
            tc.tile_pool(name="psum_seg", bufs=3, space="PSUM") as psum_seg,
            tc.tile_pool(name="psum_h", bufs=3, space="PSUM") as psum_h,
            tc.tile_pool(name="psum_tp", bufs=2, space="PSUM") as psum_tp,
            tc.tile_pool(name="dram", bufs=1, space="DRAM") as dram,
        ):
            # ---- constants ----
            rhs1_s = constp.tile([IN_C, GW], dt.bfloat16)
            nc.sync.dma_start(out=rhs1_s[:], in_=rhs1[:])
            rhs1d_s = constp.tile([IN_C, HEADS], dt.bfloat16)
            nc.sync.dma_start(out=rhs1d_s[:], in_=rhs1d[:])
            rhs2_s = constp.tile([128, 2 * PACK], dt.bfloat16)
            nc.sync.dma_start(
                out=rhs2_s[:].rearrange("p (k f) -> p k f", k=2),
                in_=rhs2[:].rearrange("(k p) f -> p k f", p=128))
            rhs3_s = constp.tile([128, 2 * (OUT_C + 2)], dt.bfloat16)
            nc.sync.dma_start(
                out=rhs3_s[:].rearrange("p (k f) -> p k f", k=2),
                in_=rhs3[:].rearrange("(k p) f -> p k f", p=128))
            b1_s = constp.tile([128, H2], dt.float32)
            nc.sync.dma_start(out=b1_s[:], in_=b1r[:])
            b2_s = constp.tile([128, H2], dt.float32)
            nc.sync.dma_start(out=b2_s[:], in_=b2r[:])
            b3_s = constp.tile([128, OUT_C], dt.float32)
            nc.sync.dma_start(out=b3_s[:], in_=b3r[:])
            iota_s = constp.tile([128, 128], dt.bfloat16)
            nc.sync.dma_start(out=iota_s[:], in_=iota[:])
            ident = constp.tile([128, 128], dt.float32)
            make_identity(nc, ident[:])
            # per-layer per-tile a_dst scores, staged in SBUF then bulk-copied
            # to the DRAM gather tables between phases
            adst_sb1 = constp.tile([128, ntiles * HEADS], dt.bfloat16)
            adst_sb2 = constp.tile([128, ntiles * HEADS], dt.bfloat16)
            adst_sb3 = constp.tile([128, ntiles], dt.bfloat16)
            # x kept SBUF-resident (in two halves) so h1 issues no
            # per-tile loads
            NTH = NTT // 2  # table tiles per half
            xhalf = constp.tile([IN_C, NTH * 128], dt.bfloat16)
            xo_res = constp.tile([IN_C, nmax], dt.bfloat16)
            nc.sync.dma_start(out=xo_res[:], in_=x_own[:])

            # ---- DRAM temporaries ----
            # loc tiles are full gather-row width so the AllGather can write
            # the wide tables directly (strided collective outs are rejected
            # by the BIR verifier, and a narrow AG + local repack costs more)
            loc12 = dram.tile([nmax, TW], dt.bfloat16)
            loc3 = dram.tile([nmax, TW3], dt.bfloat16)
            adl1 = dram.tile([nmax, TW3], dt.bfloat16)
            adl2 = dram.tile([nmax, TW3], dt.bfloat16)
            adl3 = dram.tile([nmax, TW3], dt.bfloat16)

            FCH = 13  # a_dst flush chunk, in tiles
            adfl_stage = constp.tile([128, FCH * TW3], dt.bfloat16)

            def flush_adst(adst_sb, adl, nh):
                # stage [p, t*8+h] scores into full 256B p-major rows, then
                # DMA contiguous per-partition runs (cheap descriptors)
                adlv = adl[:].rearrange("(p t) w -> p t w", t=ntiles)
                for c0 in range(0, ntiles, FCH):
                    n = min(FCH, ntiles - c0)
                    nc.vector.tensor_copy(
                        adfl_stage[:].rearrange(
                            "p (t w) -> p t w", w=TW3)[:, :n, :nh],
                        adst_sb[:, c0 * nh:(c0 + n) * nh].rearrange(
                            "p (t h) -> p t h", h=nh))
                    nc.sync.dma_start(
                        out=adlv[:, c0:c0 + n, :].rearrange(
                            "p t w -> p (t w)"),
                        in_=adfl_stage[:, :n * TW3])

            def pack12(ps, local, adst_sb, t):
                # adst_sb is the NEXT layer's table (never the one being
                # read by the current aggregate phase)
                pk = epil.tile([128, GW], dt.bfloat16, tag="pack")
                nc.scalar.copy(out=pk[:], in_=ps[:, :GW])
                nc.sync.dma_start(out=local[t * 128:(t + 1) * 128, :GW],
                                  in_=pk[:])
                nc.scalar.copy(out=adst_sb[:, t * HEADS:(t + 1) * HEADS],
                               in_=ps[:, GW:GW + HEADS])

            def pack3(ps, t):
                pk = epil.tile([128, GW3], dt.bfloat16, tag="pack")
                nc.scalar.copy(out=pk[:], in_=ps[:, :GW3])
                nc.sync.dma_start(out=loc3[t * 128:(t + 1) * 128, :GW3],
                                  in_=pk[:])
                nc.scalar.copy(out=adst_sb3[:, t:t + 1],
                               in_=ps[:, GW3:GW3 + 1])

            PKB = 7  # table tiles per batched pack write

            def h1_phase(tab1):
                # pass A: a_dst rows for own nodes
                for t in range(ntiles):
                    ps = psum_h.tile([128, PACK], dt.float32, tag="hps")
                    nc.tensor.matmul(ps[:, :HEADS],
                                     lhsT=xo_res[:, t * 128:(t + 1) * 128],
                                     rhs=rhs1d_s[:], start=True, stop=True)
                    nc.scalar.copy(out=adst_sb1[:, t * HEADS:(t + 1) * HEADS],
                                   in_=ps[:, :HEADS])
                flush_adst(adst_sb1, adl1, HEADS)
                # pass B: full [h | a_src] table, every core identically;
                # pack casts alternate ACT/DVE, table writes batched 7 tiles
                for half in range(2):
                    nc.sync.dma_start(
                        out=xhalf[:],
                        in_=xT[:, half * NTH * 128:(half + 1) * NTH * 128])
                    for G in range(NTH // PKB):
                        stage = epil.tile([128, PKB * GW], dt.bfloat16,
                                          tag="hstage")
                        for j in range(PKB):
                            Tl = G * PKB + j
                            ps = psum_h.tile([128, PACK], dt.float32,
                                             tag="hps")
                            nc.tensor.matmul(
                                ps[:, :GW],
                                lhsT=xhalf[:, Tl * 128:(Tl + 1) * 128],
                                rhs=rhs1_s[:], start=True, stop=True)
                            dstc = stage[:, j * GW:(j + 1) * GW]
                            if j % 2 == 0:
                                nc.scalar.copy(out=dstc, in_=ps[:, :GW])
                            else:
                                nc.vector.tensor_copy(dstc, ps[:, :GW])
                        r0 = (half * NTH + G * PKB) * 128
                        nc.sync.dma_start(
                            out=tab1[r0:r0 + PKB * 128, :GW].rearrange(
                                "(j p) w -> p j w", p=128),
                            in_=stage[:].rearrange("p (j w) -> p j w", w=GW))

            def allgather(local, table):
                nc.gpsimd.collective_compute(
                    "AllGather", Alu.bypass,
                    replica_groups=[list(range(NCORES))],
                    ins=[local[:].opt()], outs=[table[:].opt()])

            def epilogue12(t, ps, rhs_next_s, b_s, layer):
                # self-loops make every real node's denominator > 0; pad
                # nodes produce inf/NaN rows that are never read
                recip = epil.tile([128, HEADS], dt.float32, tag="recip")
                nc.vector.reciprocal(recip[:], ps[:, H2:H2 + HEADS])
                act = epil.tile([128, H2], dt.float32, tag="act")
                nc.vector.tensor_tensor(
                    out=act[:].rearrange("p (c h) -> p c h", h=HEADS),
                    in0=ps[:, :H2].rearrange("p (c h) -> p c h", h=HEADS),
                    in1=recip[:].unsqueeze(1).to_broadcast([128, HID, HEADS]),
                    op=Alu.mult)
                nc.vector.tensor_add(out=act[:], in0=act[:], in1=b_s[:])
                nc.scalar.activation(out=act[:], in_=act[:], func=Act.Relu)
                w = PACK if layer == 1 else OUT_C + 2
                hps = psum_h.tile([128, PACK], dt.float32, tag="hps")
                for kc in range(2):
                    tp = psum_tp.tile([128, 128], dt.float32, tag="tp")
                    nc.tensor.transpose(
                        out=tp[:], in_=act[:, kc * 128:(kc + 1) * 128],
                        identity=ident[:])
                    aT = epil.tile([128, 128], dt.bfloat16, tag="aT")
                    nc.scalar.copy(out=aT[:], in_=tp[:])
                    nc.tensor.matmul(
                        hps[:, :w], lhsT=aT[:],
                        rhs=rhs_next_s[:, kc * w:(kc + 1) * w],
                        start=(kc == 0), stop=(kc == 1))
                if layer == 1:
                    pack12(hps, loc12, adst_sb2, t)
                else:
                    pack3(hps, t)

            def epilogue3(t, ps):
                recip = epil.tile([128, 1], dt.float32, tag="recip3")
                nc.vector.reciprocal(recip[:], ps[:, OUT_C:OUT_C + 1])
                o3 = epil.tile([128, OUT_C], dt.float32, tag="o3")
                nc.vector.tensor_scalar(
                    out=o3[:], in0=ps[:, :OUT_C], scalar1=recip[:, :1],
                    scalar2=None, op0=Alu.mult)
                nc.vector.tensor_add(out=o3[:], in0=o3[:], in1=b3_s[:])
                mneg = epil.tile([128, 1], dt.float32, tag="mneg")
                nc.vector.tensor_reduce(
                    out=mneg[:], in_=o3[:], axis=mybir.AxisListType.X,
                    op=Alu.max, negate=True)
                es = epil.tile([128, OUT_C], dt.float32, tag="es")
                ssum = epil.tile([128, 1], dt.float32, tag="ssum")
                nc.scalar.activation(out=es[:], in_=o3[:], func=Act.Exp,
                                     bias=mneg[:, :1], accum_out=ssum[:, :1])
                lse = epil.tile([128, 1], dt.float32, tag="lse")
                nc.scalar.activation(out=lse[:], in_=ssum[:], func=Act.Ln)
                fin = epil.tile([128, OUT_C], dt.float32, tag="fin")
                nc.vector.tensor_scalar(
                    out=fin[:], in0=o3[:], scalar1=mneg[:, :1],
                    scalar2=lse[:, :1], op0=Alu.add, op1=Alu.subtract)
                nc.sync.dma_start(out=out[t * 128:(t + 1) * 128, :], in_=fin[:])

            def aggregate(layer, table, adl, rhs_next_s, b_s):
                if layer == 3:
                    gw, nfeat, nh, tw = GW3, OUT_C, 1, TW3
                else:
                    gw, nfeat, nh, tw = GW, H2, HEADS, TW
                KH = KSUP // 2

                def run_epilogue(t, ps):
                    if no_epil:
                        return
                    if layer == 3:
                        epilogue3(t, ps)
                    else:
                        epilogue12(t, ps, rhs_next_s, b_s, layer)

                ps_cur = None
                pending = []  # epilogues deferred one sup so their waits
                # never stall the engine queues mid-pipeline
                for sup in range(nsup):
                    for (t, ps) in pending:
                        run_epilogue(t, ps)
                    pending = []
                    idxt = sbuf.tile([128, BLK], dt.int16, tag="idxt")
                    nc.sync.dma_start(
                        out=idxt[:],
                        in_=idxs_in[:, sup * BLK:(sup + 1) * BLK])
                    sidx = idxt[:, 0:SB]
                    didx = idxt[:, SB:2 * SB]
                    segt = idxt[:, 2 * SB:BLK].bitcast(dt.float32)

                    gt = gbuf.tile([128, KSUP, tw], dt.bfloat16,
                                   tag="g3" if layer == 3 else "gt")
                    if not no_srcg:
                        for rr, (a, b, tag) in enumerate(runs_by_sup[sup]):
                            a0, b0 = a - sup * KSUP, b - sup * KSUP
                            nidx = (b - a) * 128
                            src_ap = (table[:half, :] if tag == 0
                                      else table[half:2 * half, :])
                            nc.gpsimd.dma_gather(
                                out_ap=gt[:, a0:b0, :], in_ap=src_ap,
                                idxs_ap=sidx[:, a0 * 8:b0 * 8],
                                num_idxs=nidx, num_idxs_reg=nidx,
                                elem_size=tw,
                                queue_num=(sup + rr) % 2)
                    dts = gbuf.tile([128, KSUP, TW3], dt.bfloat16, tag="dts")
                    if not no_adst:
                        nc.gpsimd.dma_gather(
                            out_ap=dts[:], in_ap=adl[:], idxs_ap=didx[:],
                            num_idxs=KSUP * 128, num_idxs_reg=KSUP * 128,
                            elem_size=TW3, queue_num=2 + sup % 2)

                    mt = None
                    if not no_mm:
                        mt = mbuf.tile([128, KSUP * 128], dt.bfloat16,
                                       tag="mt")
                        for kk in range(KSUP):
                            nc.vector.tensor_scalar(
                                out=mt[:, kk * 128:(kk + 1) * 128],
                                in0=iota_s[:],
                                scalar1=segt[:, kk:kk + 1], scalar2=None,
                                op0=Alu.is_equal)

                    if not no_vec:
                        wt = gbuf.tile([128, KSUP, nh], dt.bfloat16, tag="wt")
                        in1 = (gt[:, :, nfeat:nfeat + nh]
                               if (no_adst or no_srcg) else dts[:, :, :nh])
                        nc.vector.tensor_tensor(
                            out=wt[:], in0=gt[:, :, nfeat:nfeat + nh],
                            in1=in1, op=Alu.add)
                        nc.scalar.activation(out=wt[:], in_=wt[:],
                                             func=Act.Prelu, alpha=NEG_SLOPE)
                        nc.scalar.activation(out=wt[:], in_=wt[:],
                                             func=Act.Exp)
                        # message scaling split in chunk-halves so the first
                        # segment matmuls can start while the second half is
                        # still on DVE
                        for hh in range(2):
                            ksl = slice(hh * KH, (hh + 1) * KH)
                            if layer != 3:
                                nc.vector.tensor_tensor(
                                    out=gt[:, ksl, :nfeat].rearrange(
                                        "p k (c h) -> p k c h", h=HEADS),
                                    in0=gt[:, ksl, :nfeat].rearrange(
                                        "p k (c h) -> p k c h", h=HEADS),
                                    in1=wt[:, ksl].unsqueeze(2).to_broadcast(
                                        [128, KH, HID, HEADS]),
                                    op=Alu.mult)
                            else:
                                nc.vector.tensor_tensor(
                                    out=gt[:, ksl, :nfeat],
                                    in0=gt[:, ksl, :nfeat],
                                    in1=wt[:, ksl].to_broadcast(
                                        [128, KH, nfeat]),
                                    op=Alu.mult)
                            nc.vector.tensor_copy(
                                gt[:, ksl, nfeat:nfeat + nh], wt[:, ksl])

                    if no_mm:
                        continue
                    for kk in range(KSUP):
                        q = sup * KSUP + kk
                        t = int(tile_of_chunk[q])
                        if q == first_chunk[t]:
                            ps_cur = psum_seg.tile([128, GW], dt.float32,
                                                   tag="segps")
                        nc.tensor.matmul(
                            ps_cur[:, :gw],
                            lhsT=mt[:, kk * 128:(kk + 1) * 128],
                            rhs=gt[:, kk, :gw],
                            start=(q == first_chunk[t]),
                            stop=(q == last_chunk[t]))
                        if q == last_chunk[t]:
                            pending.append((t, ps_cur))
                for (t, ps) in pending:
                    run_epilogue(t, ps)

            for _rep in range(repeat):
                tab1 = dram.tile([NCORES * nmax, TW], dt.bfloat16,
                                 name=f"tab1_{_rep}")
                tab2 = dram.tile([NCORES * nmax, TW], dt.bfloat16,
                                 addr_space="Shared", name=f"tab2_{_rep}")
                tab3 = dram.tile([NCORES * nmax, TW3], dt.bfloat16,
                                 addr_space="Shared", name=f"tab3_{_rep}")
                h1_phase(tab1)
                if nphase >= 1:
                    aggregate(1, tab1, adl1, rhs2_s, b1_s)
                if nphase >= 2:
                    flush_adst(adst_sb2, adl2, HEADS)
                    allgather(loc12, tab2)
                    aggregate(2, tab2, adl2, rhs3_s, b2_s)
                if nphase >= 3:
                    flush_adst(adst_sb3, adl3, 1)
                    allgather(loc3, tab3)
                    aggregate(3, tab3, adl3, None, None)

    nc.compile()
    return nc


def _make_in_maps(x, g, wts):
    """Per-core input dicts. x: [N, IN_C] f32; wts: _prep_weights output."""
    rhs1, rhs1d, rhs2, rhs3, b1r, b2r, b3r = wts
    npc, nmax = g["npc"], g["nmax"]
    iota = np.tile(np.arange(128, dtype=np.float32)[None, :],
                   (128, 1)).astype(_BF16)
    xTf = np.zeros((IN_C, NCORES * nmax), _BF16)
    for k in range(NCORES):
        xTf[:, k * nmax:k * nmax + npc] = x[k * npc:(k + 1) * npc].T
    in_maps = []
    for k in range(NCORES):
        in_maps.append({
            "xT": xTf, "x_own": xTf[:, k * nmax:(k + 1) * nmax],
            "rhs1": rhs1, "rhs1d": rhs1d, "rhs2": rhs2, "rhs3": rhs3,
            "b1r": b1r, "b2r": b2r, "b3r": b3r, "iota": iota,
            "idxs": np.ascontiguousarray(g["idxc"][k]).reshape(
                128, -1),
        })
    return in_maps


_CACHE = {}


def kernel(x, edge_index, W1, as1, ad1, b1, W2, as2, ad2, b2, W3, as3, ad3, b3,
           _repeat=1):
    from concourse.bass_utils import run_bass_kernel_spmd

    x = np.asarray(x, np.float32)
    edge_index = np.asarray(edge_index)
    g = _prep_graph(edge_index)
    wts = _prep_weights(W1, as1, ad1, b1, W2, as2, ad2, b2, W3, as3, ad3, b3)

    key = (hash(edge_index.tobytes()), _repeat)
    if key not in _CACHE:
        _CACHE[key] = _build_bass(g, repeat=_repeat)
    nc = _CACHE[key]

    in_maps = _make_in_maps(x, g, wts)
    res = run_bass_kernel_spmd(nc, in_maps, core_ids=list(range(NCORES)))
    npc = g["npc"]
    outf = np.zeros((N, OUT_C), np.float32)
    for k in range(NCORES):
        outf[k * npc:(k + 1) * npc] = res.results[k]["out"][:npc]
    return outf


# revision 91
# speedup vs baseline: 1.0831x; 1.0104x over previous
"""3-layer GAT on 8 Trainium2 NeuronCores (Bass/Tile).

Edge-sharded by destination range:
  - Nodes split into 8 contiguous ranges (one per core); each core owns the
    softmax + aggregation for its destination nodes.
  - Layer 1's per-node table [h | a_src] is computed LOCALLY on every core
    from a replicated copy of x (no collective): x is an input, so each core
    can build the full 50176-row table with 392 small bf16 matmuls, which is
    much cheaper than the 38MB AllGather it replaces.  Per-core a_dst rows
    come from a second tiny matmul pass over the core's own x shard.
  - For layers 2/3 the aggregation output is only known by the dst-owning
    core, so tables are AllGathered at full gather-row width (768B/256B
    rows) straight into the tables: collectives reject strided outs, and a
    narrow AG + strided repack is far slower on real HW than the extra
    collective bytes.
  - Edges (with self loops) are bucketed per core into 128-dst tiles x
    128-edge chunks; chunk structure (incl. lo/hi int16-index table halves)
    is made identical across cores so one SPMD instruction stream fits all.
  - Per 8-chunk super-batch the kernel dma_gathers source rows + dest
    attention rows, computes w = exp(leaky_relu(a_src+a_dst)) (softmax
    shift-invariance removes the segment-max pass at these value ranges),
    scales messages by w, and segment-sums with matmuls against one-hot
    membership matrices, keeping numerator and denominator together in
    PSUM.  The per-tile epilogue divides, applies bias/relu, and feeds the
    next layer's matmul whose rhs [W | W@att_src | W@att_dst] also emits
    the next attention scores.
"""

import numpy as np
import ml_dtypes

N = 50000
E = 800000
IN_C = 128
HID = 32
OUT_C = 40
HEADS = 8
NEG_SLOPE = 0.2
NCORES = 8

_BF16 = ml_dtypes.bfloat16

KSUP = 8  # chunks per gather super-batch (1024-idx dma_gather limit)
SB = KSUP * 8  # int16 idx cols per sup for one index stream
BLK = 2 * SB + 2 * KSUP  # per-sup cols: sidx | didx | seg bytes


def _cmajor_perm(heads, ch):
    f_new = np.arange(heads * ch)
    return (f_new % heads) * ch + f_new // heads  # perm[new] = old


def _attn_cols(w, att):
    heads, ch = att.shape
    return np.einsum("khc,hc->kh", w.reshape(-1, heads, ch), att).astype(np.float32)


def _prep_weights(W1, as1, ad1, b1, W2, as2, ad2, b2, W3, as3, ad3, b3):
    W1 = np.asarray(W1, np.float32)
    W2 = np.asarray(W2, np.float32)
    W3 = np.asarray(W3, np.float32)
    perm = _cmajor_perm(HEADS, HID)

    rhs1 = np.concatenate(
        [W1[:, perm], _attn_cols(W1, np.asarray(as1, np.float32))],
        axis=1).astype(_BF16)
    rhs1d = _attn_cols(W1, np.asarray(ad1, np.float32)).astype(_BF16)
    W2r = W2[perm, :]
    rhs2 = np.concatenate(
        [W2r[:, perm], _attn_cols(W2r, np.asarray(as2, np.float32)),
         _attn_cols(W2r, np.asarray(ad2, np.float32))], axis=1).astype(_BF16)
    W3r = W3[perm, :]
    as3p = (W3r @ np.asarray(as3, np.float32)[0]).reshape(-1, 1)
    ad3p = (W3r @ np.asarray(ad3, np.float32)[0]).reshape(-1, 1)
    rhs3 = np.concatenate([W3r, as3p, ad3p], axis=1).astype(_BF16)

    def bcast(b):
        return np.tile(np.asarray(b, np.float32)[None, :], (128, 1))

    return (rhs1, rhs1d, rhs2, rhs3,
            bcast(np.asarray(b1, np.float32)[perm]),
            bcast(np.asarray(b2, np.float32)[perm]),
            bcast(np.asarray(b3, np.float32)))


def _prep_graph(edge_index):
    """Slot edges into the SPMD-uniform (tile, section, chunk) grid."""
    src = np.concatenate([edge_index[0], np.arange(N)]).astype(np.int64)
    dst = np.concatenate([edge_index[1], np.arange(N)]).astype(np.int64)

    npc = N // NCORES
    ntiles = (npc + 127) // 128
    nmax = ntiles * 128
    half = (NCORES // 2) * nmax

    core_of = dst // npc
    d_loc = dst - core_of * npc
    tile_of = d_loc // 128
    s_core = src // npc
    s_row = s_core * nmax + (src - s_core * npc)  # table row of src
    is_hi = s_row >= half

    cnt = np.zeros((NCORES, ntiles, 2), np.int64)
    np.add.at(cnt, (core_of, tile_of, is_hi.astype(np.int64)), 1)
    sec_cpt = np.ceil(cnt / 128).astype(np.int64).max(axis=0)  # [ntiles, 2]
    sec_cpt[:, 0] = np.maximum(sec_cpt[:, 0], 1)

    total = int(sec_cpt.sum())
    pad = (-total) % KSUP
    sec_cpt[-1, 1] += pad
    total += pad
    nsup = total // KSUP

    tile_of_chunk = []
    tag_of_chunk = []
    for t in range(ntiles):
        tile_of_chunk += [t] * int(sec_cpt[t, 0] + sec_cpt[t, 1])
        tag_of_chunk += [0] * int(sec_cpt[t, 0]) + [1] * int(sec_cpt[t, 1])
    tile_of_chunk = np.array(tile_of_chunk)
    tag_of_chunk = np.array(tag_of_chunk)
    sec_base = np.zeros((ntiles, 2), np.int64)
    sec_base.ravel()[1:] = np.cumsum(sec_cpt.ravel())[:-1]

    # combined per-sup stream: [sidx | didx | seg-bytes] int16 cols
    idxc = np.zeros((NCORES, 128, nsup, BLK), np.int16)
    seg = np.full((NCORES, nsup, 128, KSUP), 255.0, np.float32)

    order = np.lexsort((src, is_hi, tile_of, core_of))
    src_o = s_row[order]
    dst_o = d_loc[order]
    core_o = core_of[order]
    tile_o = tile_of[order]
    hi_o = is_hi[order]

    for k in range(NCORES):
        m = core_o == k
        t = tile_o[m]
        hi = hi_o[m].astype(np.int64)
        sr = src_o[m] - hi * half
        dl = dst_o[m]
        key = t * 2 + hi
        cnts = np.bincount(key, minlength=ntiles * 2)
        st = np.zeros(ntiles * 2, np.int64)
        st[1:] = np.cumsum(cnts)[:-1]
        pos = np.arange(len(t)) - st[key]
        q = sec_base[t, hi] + pos // 128
        p = pos % 128
        colsup = q // KSUP
        col = (q % KSUP) * 8 + p // 16
        row = p % 16
        # a_dst table rows are p-major (row = (d%128)*ntiles + d//128) so
        # the SBUF->DRAM flush is 128 contiguous runs instead of 6k tiny ones
        dlp = (dl % 128) * ntiles + dl // 128
        for c in range(8):
            idxc[k, row + 16 * c, colsup, col] = sr
            idxc[k, row + 16 * c, colsup, SB + col] = dlp
        seg[k, q // KSUP, p, q % KSUP] = (dl % 128).astype(np.float32)
    for k in range(NCORES):
        idxc[k, :, :, 2 * SB:] = np.ascontiguousarray(
            seg[k].transpose(1, 0, 2)).view(np.int16).reshape(
            128, nsup, 2 * KSUP)

    runs = []  # (sup, chunk_lo, chunk_hi, tag)
    for s in range(nsup):
        q0 = s * KSUP
        r0 = q0
        for q in range(q0 + 1, q0 + KSUP + 1):
            if q == q0 + KSUP or tag_of_chunk[q] != tag_of_chunk[r0]:
                runs.append((s, r0, q, int(tag_of_chunk[r0])))
                r0 = q

    return dict(
        idxc=idxc,
        tile_of_chunk=tile_of_chunk, runs=runs, nsup=nsup, total=total,
        ntiles=ntiles, nmax=nmax, npc=npc, half=half,
    )


def _build_bass(g, repeat=1):
    import os
    import concourse.bacc as bacc
    import concourse.mybir as mybir
    import concourse.tile as tile
    from concourse.masks import make_identity

    dt = mybir.dt
    Alu = mybir.AluOpType
    Act = mybir.ActivationFunctionType

    ntiles, nmax, nsup, total = g["ntiles"], g["nmax"], g["nsup"], g["total"]
    half = g["half"]
    tile_of_chunk = g["tile_of_chunk"]
    H2 = HEADS * HID  # 256
    GW = H2 + HEADS  # 264 useful table cols: h + a_src
    PACK = GW + HEADS  # 272 psum width in epilogue: h + a_src + a_dst
    TW = 384  # gather-table row width (768B)
    TW3 = 128  # layer-3 / a_dst table row width (256B)
    GW3 = OUT_C + 1  # 41
    NTT = NCORES * ntiles  # total table tiles

    first_chunk = {}
    last_chunk = {}
    for q, t in enumerate(tile_of_chunk):
        first_chunk.setdefault(int(t), q)
        last_chunk[int(t)] = q
    runs_by_sup = {}
    for (s, a, b, tag) in g["runs"]:
        runs_by_sup.setdefault(s, []).append((a, b, tag))

    nphase = int(os.environ.get("GAT_PHASES", "3"))
    # timing-only knock-outs (break correctness; for bottleneck isolation)
    no_srcg = int(os.environ.get("GAT_NO_SRCG", "0"))
    no_adst = int(os.environ.get("GAT_NO_ADST", "0"))
    no_vec = int(os.environ.get("GAT_NO_VEC", "0"))
    no_mm = int(os.environ.get("GAT_NO_MM", "0"))
    no_epil = int(os.environ.get("GAT_NO_EPIL", "0"))
    gbufs = int(os.environ.get("GAT_GBUFS", "8"))

    nc = bacc.Bacc("TRN2", target_bir_lowering=False, debug=False,
                   num_devices=NCORES, num_swdge_queues=4)

    xT = nc.dram_tensor("xT", [IN_C, NCORES * nmax], dt.bfloat16,
                        kind="ExternalInput")
    x_own = nc.dram_tensor("x_own", [IN_C, nmax], dt.bfloat16,
                           kind="ExternalInput")
    rhs1 = nc.dram_tensor("rhs1", [IN_C, GW], dt.bfloat16,
                          kind="ExternalInput")
    rhs1d = nc.dram_tensor("rhs1d", [IN_C, HEADS], dt.bfloat16,
                           kind="ExternalInput")
    rhs2 = nc.dram_tensor("rhs2", [H2, PACK], dt.bfloat16,
                          kind="ExternalInput")
    rhs3 = nc.dram_tensor("rhs3", [H2, OUT_C + 2], dt.bfloat16,
                          kind="ExternalInput")
    b1r = nc.dram_tensor("b1r", [128, H2], dt.float32, kind="ExternalInput")
    b2r = nc.dram_tensor("b2r", [128, H2], dt.float32, kind="ExternalInput")
    b3r = nc.dram_tensor("b3r", [128, OUT_C], dt.float32, kind="ExternalInput")
    iota = nc.dram_tensor("iota", [128, 128], dt.bfloat16, kind="ExternalInput")
    idxs_in = nc.dram_tensor("idxs", [128, nsup * BLK], dt.int16,
                             kind="ExternalInput")
    out = nc.dram_tensor("out", [nmax, OUT_C], dt.float32,
                         kind="ExternalOutput")

    with tile.TileContext(nc) as tc:
        with (
            tc.tile_pool(name="const", bufs=1) as constp,
            tc.tile_pool(name="sbuf", bufs=6) as sbuf,
            tc.tile_pool(name="gbuf", bufs=gbufs) as gbuf,
            tc.tile_pool(name="mbuf", bufs=6) as mbuf,
            tc.tile_pool(name="epil", bufs=2) as epil,
            tc.tile_pool(name="psum_seg", bufs=3, space="PSUM") as psum_seg,
            tc.tile_pool(name="psum_h", bufs=3, space="PSUM") as psum_h,
            tc.tile_pool(name="psum_tp", bufs=2, space="PSUM") as psum_tp,
            tc.tile_pool(name="dram", bufs=1, space="DRAM") as dram,
        ):
            # ---- constants ----
            rhs1_s = constp.tile([IN_C, GW], dt.bfloat16)
            nc.sync.dma_start(out=rhs1_s[:], in_=rhs1[:])
            rhs1d_s = constp.tile([IN_C, HEADS], dt.bfloat16)
            nc.sync.dma_start(out=rhs1d_s[:], in_=rhs1d[:])
            rhs2_s = constp.tile([128, 2 * PACK], dt.bfloat16)
            nc.sync.dma_start(
                out=rhs2_s[:].rearrange("p (k f) -> p k f", k=2),
                in_=rhs2[:].rearrange("(k p) f -> p k f", p=128))
            rhs3_s = constp.tile([128, 2 * (OUT_C + 2)], dt.bfloat16)
            nc.sync.dma_start(
                out=rhs3_s[:].rearrange("p (k f) -> p k f", k=2),
                in_=rhs3[:].rearrange("(k p) f -> p k f", p=128))
            b1_s = constp.tile([128, H2], dt.float32)
            nc.sync.dma_start(out=b1_s[:], in_=b1r[:])
            b2_s = constp.tile([128, H2], dt.float32)
            nc.sync.dma_start(out=b2_s[:], in_=b2r[:])
            b3_s = constp.tile([128, OUT_C], dt.float32)
            nc.sync.dma_start(out=b3_s[:], in_=b3r[:])
            iota_s = constp.tile([128, 128], dt.bfloat16)
            nc.sync.dma_start(out=iota_s[:], in_=iota[:])
            ident = constp.tile([128, 128], dt.float32)
            make_identity(nc, ident[:])
            # per-layer per-tile a_dst scores, staged in SBUF then bulk-copied
            # to the DRAM gather tables between phases
            adst_sb1 = constp.tile([128, ntiles * HEADS], dt.bfloat16)
            adst_sb2 = constp.tile([128, ntiles * HEADS], dt.bfloat16)
            adst_sb3 = constp.tile([128, ntiles], dt.bfloat16)
            # x kept SBUF-resident (in two halves) so h1 issues no
            # per-tile loads
            NTH = NTT // 2  # table tiles per half
            xhalf = constp.tile([IN_C, NTH * 128], dt.bfloat16)
            xo_res = constp.tile([IN_C, nmax], dt.bfloat16)
            nc.sync.dma_start(out=xo_res[:], in_=x_own[:])

            # ---- DRAM temporaries ----
            # loc tiles are full gather-row width so the AllGather can write
            # the wide tables directly (strided collective outs are rejected
            # by the BIR verifier, and a narrow AG + local repack costs more)
            loc12 = dram.tile([nmax, TW], dt.bfloat16)
            loc3 = dram.tile([nmax, TW3], dt.bfloat16)
            adl1 = dram.tile([nmax, TW3], dt.bfloat16)
            adl2 = dram.tile([nmax, TW3], dt.bfloat16)
            adl3 = dram.tile([nmax, TW3], dt.bfloat16)

            FCH = 13  # a_dst flush chunk, in tiles
            adfl_stage = constp.tile([128, FCH * TW3], dt.bfloat16)

            def flush_adst(adst_sb, adl, nh):
                # stage [p, t*8+h] scores into full 256B p-major rows, then
                # DMA contiguous per-partition runs (cheap descriptors)
                adlv = adl[:].rearrange("(p t) w -> p t w", t=ntiles)
                for c0 in range(0, ntiles, FCH):
                    n = min(FCH, ntiles - c0)
                    nc.vector.tensor_copy(
                        adfl_stage[:].rearrange(
                            "p (t w) -> p t w", w=TW3)[:, :n, :nh],
                        adst_sb[:, c0 * nh:(c0 + n) * nh].rearrange(
                            "p (t h) -> p t h", h=nh))
                    nc.sync.dma_start(
                        out=adlv[:, c0:c0 + n, :].rearrange(
                            "p t w -> p (t w)"),
                        in_=adfl_stage[:, :n * TW3])

            def pack12(ps, local, adst_sb, t):
                # adst_sb is the NEXT layer's table (never the one being
                # read by the current aggregate phase)
                pk = epil.tile([128, GW], dt.bfloat16, tag="pack")
                nc.scalar.copy(out=pk[:], in_=ps[:, :GW])
                nc.sync.dma_start(out=local[t * 128:(t + 1) * 128, :GW],
                                  in_=pk[:])
                nc.scalar.copy(out=adst_sb[:, t * HEADS:(t + 1) * HEADS],
                               in_=ps[:, GW:GW + HEADS])

            def pack3(ps, t):
                pk = epil.tile([128, GW3], dt.bfloat16, tag="pack")
                nc.scalar.copy(out=pk[:], in_=ps[:, :GW3])
                nc.sync.dma_start(out=loc3[t * 128:(t + 1) * 128, :GW3],
                                  in_=pk[:])
                nc.scalar.copy(out=adst_sb3[:, t:t + 1],
                               in_=ps[:, GW3:GW3 + 1])

            PKB = 7  # table tiles per batched pack write

            def h1_phase(tab1):
                # pass A: a_dst rows for own nodes
                for t in range(ntiles):
                    ps = psum_h.tile([128, PACK], dt.float32, tag="hps")
                    nc.tensor.matmul(ps[:, :HEADS],
                                     lhsT=xo_res[:, t * 128:(t + 1) * 128],
                                     rhs=rhs1d_s[:], start=True, stop=True)
                    nc.scalar.copy(out=adst_sb1[:, t * HEADS:(t + 1) * HEADS],
                                   in_=ps[:, :HEADS])
                flush_adst(adst_sb1, adl1, HEADS)
                # pass B: full [h | a_src] table, every core identically;
                # pack casts alternate ACT/DVE, table writes batched 7 tiles
                for half in range(2):
                    nc.sync.dma_start(
                        out=xhalf[:],
                        in_=xT[:, half * NTH * 128:(half + 1) * NTH * 128])
                    for G in range(NTH // PKB):
                        stage = epil.tile([128, PKB * GW], dt.bfloat16,
                                          tag="hstage")
                        for j in range(PKB):
                            Tl = G * PKB + j
                            ps = psum_h.tile([128, PACK], dt.float32,
                                             tag="hps")
                            nc.tensor.matmul(
                                ps[:, :GW],
                                lhsT=xhalf[:, Tl * 128:(Tl + 1) * 128],
                                rhs=rhs1_s[:], start=True, stop=True)
                            dstc = stage[:, j * GW:(j + 1) * GW]
                            if j % 2 == 0:
                                nc.scalar.copy(out=dstc, in_=ps[:, :GW])
                            else:
                                nc.vector.tensor_copy(dstc, ps[:, :GW])
                        r0 = (half * NTH + G * PKB) * 128
                        nc.sync.dma_start(
                            out=tab1[r0:r0 + PKB * 128, :GW].rearrange(
                                "(j p) w -> p j w", p=128),
                            in_=stage[:].rearrange("p (j w) -> p j w", w=GW))

            def allgather(local, table):
                nc.gpsimd.collective_compute(
                    "AllGather", Alu.bypass,
                    replica_groups=[list(range(NCORES))],
                    ins=[local[:].opt()], outs=[table[:].opt()])

            def epilogue12(t, ps, rhs_next_s, b_s, layer):
                # self-loops make every real node's denominator > 0; pad
                # nodes produce inf/NaN rows that are never read
                recip = epil.tile([128, HEADS], dt.float32, tag="recip")
                nc.vector.reciprocal(recip[:], ps[:, H2:H2 + HEADS])
                act = epil.tile([128, H2], dt.float32, tag="act")
                nc.vector.tensor_tensor(
                    out=act[:].rearrange("p (c h) -> p c h", h=HEADS),
                    in0=ps[:, :H2].rearrange("p (c h) -> p c h", h=HEADS),
                    in1=recip[:].unsqueeze(1).to_broadcast([128, HID, HEADS]),
                    op=Alu.mult)
                nc.vector.tensor_add(out=act[:], in0=act[:], in1=b_s[:])
                nc.scalar.activation(out=act[:], in_=act[:], func=Act.Relu)
                w = PACK if layer == 1 else OUT_C + 2
                hps = psum_h.tile([128, PACK], dt.float32, tag="hps")
                for kc in range(2):
                    tp = psum_tp.tile([128, 128], dt.float32, tag="tp")
                    nc.tensor.transpose(
                        out=tp[:], in_=act[:, kc * 128:(kc + 1) * 128],
                        identity=ident[:])
                    aT = epil.tile([128, 128], dt.bfloat16, tag="aT")
                    nc.scalar.copy(out=aT[:], in_=tp[:])
                    nc.tensor.matmul(
                        hps[:, :w], lhsT=aT[:],
                        rhs=rhs_next_s[:, kc * w:(kc + 1) * w],
                        start=(kc == 0), stop=(kc == 1))
                if layer == 1:
                    pack12(hps, loc12, adst_sb2, t)
                else:
                    pack3(hps, t)

            def epilogue3(t, ps):
                recip = epil.tile([128, 1], dt.float32, tag="recip3")
                nc.vector.reciprocal(recip[:], ps[:, OUT_C:OUT_C + 1])
                o3 = epil.tile([128, OUT_C], dt.float32, tag="o3")
                nc.vector.tensor_scalar(
                    out=o3[:], in0=ps[:, :OUT_C], scalar1=recip[:, :1],
                    scalar2=None, op0=Alu.mult)
                nc.vector.tensor_add(out=o3[:], in0=o3[:], in1=b3_s[:])
                mneg = epil.tile([128, 1], dt.float32, tag="mneg")
                nc.vector.tensor_reduce(
                    out=mneg[:], in_=o3[:], axis=mybir.AxisListType.X,
                    op=Alu.max, negate=True)
                es = epil.tile([128, OUT_C], dt.float32, tag="es")
                ssum = epil.tile([128, 1], dt.float32, tag="ssum")
                nc.scalar.activation(out=es[:], in_=o3[:], func=Act.Exp,
                                     bias=mneg[:, :1], accum_out=ssum[:, :1])
                lse = epil.tile([128, 1], dt.float32, tag="lse")
                nc.scalar.activation(out=lse[:], in_=ssum[:], func=Act.Ln)
                fin = epil.tile([128, OUT_C], dt.float32, tag="fin")
                nc.vector.tensor_scalar(
                    out=fin[:], in0=o3[:], scalar1=mneg[:, :1],
                    scalar2=lse[:, :1], op0=Alu.add, op1=Alu.subtract)
                nc.sync.dma_start(out=out[t * 128:(t + 1) * 128, :], in_=fin[:])

            def aggregate(layer, table, adl, rhs_next_s, b_s):
                if layer == 3:
                    gw, nfeat, nh, tw = GW3, OUT_C, 1, TW3
                else:
                    gw, nfeat, nh, tw = GW, H2, HEADS, TW
                KH = KSUP // 2

                def run_epilogue(t, ps):
                    if no_epil:
                        return
                    if layer == 3:
                        epilogue3(t, ps)
                    else:
                        epilogue12(t, ps, rhs_next_s, b_s, layer)

                ps_cur = None
                pending = []  # epilogues deferred one sup so their waits
                # never stall the engine queues mid-pipeline
                idxt2 = None
                for sup in range(nsup):
                    for (t, ps) in pending:
                        run_epilogue(t, ps)
                    pending = []
                    if sup % 2 == 0:
                        idxt2 = sbuf.tile([128, 2 * BLK], dt.int16,
                                          tag="idxt")
                        n2 = min(2, nsup - sup)
                        nc.sync.dma_start(
                            out=idxt2[:, :n2 * BLK],
                            in_=idxs_in[:, sup * BLK:(sup + n2) * BLK])
                    o = (sup % 2) * BLK
                    sidx = idxt2[:, o:o + SB]
                    didx = idxt2[:, o + SB:o + 2 * SB]
                    segt = idxt2[:, o + 2 * SB:o + BLK].bitcast(dt.float32)

                    gt = gbuf.tile([128, KSUP, tw], dt.bfloat16,
                                   tag="g3" if layer == 3 else "gt")
                    if not no_srcg:
                        for rr, (a, b, tag) in enumerate(runs_by_sup[sup]):
                            a0, b0 = a - sup * KSUP, b - sup * KSUP
                            nidx = (b - a) * 128
                            src_ap = (table[:half, :] if tag == 0
                                      else table[half:2 * half, :])
                            nc.gpsimd.dma_gather(
                                out_ap=gt[:, a0:b0, :], in_ap=src_ap,
                                idxs_ap=sidx[:, a0 * 8:b0 * 8],
                                num_idxs=nidx, num_idxs_reg=nidx,
                                elem_size=tw,
                                queue_num=(sup + rr) % 2)
                    dts = gbuf.tile([128, KSUP, TW3], dt.bfloat16, tag="dts")
                    if not no_adst:
                        nc.gpsimd.dma_gather(
                            out_ap=dts[:], in_ap=adl[:], idxs_ap=didx[:],
                            num_idxs=KSUP * 128, num_idxs_reg=KSUP * 128,
                            elem_size=TW3, queue_num=2 + sup % 2)

                    mt = None
                    if not no_mm:
                        mt = mbuf.tile([128, KSUP * 128], dt.bfloat16,
                                       tag="mt")
                        for kk in range(KSUP):
                            nc.vector.tensor_scalar(
                                out=mt[:, kk * 128:(kk + 1) * 128],
                                in0=iota_s[:],
                                scalar1=segt[:, kk:kk + 1], scalar2=None,
                                op0=Alu.is_equal)

                    if not no_vec:
                        # attention weights computed in-place in the gather
                        # tile's [nfeat:nfeat+nh] columns: they both scale the
                        # messages and become the denominator rhs columns
                        wcols = gt[:, :, nfeat:nfeat + nh]
                        in1 = (wcols if (no_adst or no_srcg)
                               else dts[:, :, :nh])
                        nc.vector.tensor_tensor(out=wcols, in0=wcols,
                                                in1=in1, op=Alu.add)
                        nc.scalar.activation(out=wcols, in_=wcols,
                                             func=Act.Prelu, alpha=NEG_SLOPE)
                        nc.scalar.activation(out=wcols, in_=wcols,
                                             func=Act.Exp)
                        # message scaling split in chunk-halves so the first
                        # segment matmuls can start while the second half is
                        # still on DVE
                        for hh in range(2):
                            ksl = slice(hh * KH, (hh + 1) * KH)
                            if layer != 3:
                                nc.vector.tensor_tensor(
                                    out=gt[:, ksl, :nfeat].rearrange(
                                        "p k (c h) -> p k c h", h=HEADS),
                                    in0=gt[:, ksl, :nfeat].rearrange(
                                        "p k (c h) -> p k c h", h=HEADS),
                                    in1=gt[:, ksl, nfeat:nfeat + nh]
                                    .unsqueeze(2).to_broadcast(
                                        [128, KH, HID, HEADS]),
                                    op=Alu.mult)
                            else:
                                nc.vector.tensor_tensor(
                                    out=gt[:, ksl, :nfeat],
                                    in0=gt[:, ksl, :nfeat],
                                    in1=gt[:, ksl, nfeat:nfeat + 1]
                                    .to_broadcast([128, KH, nfeat]),
                                    op=Alu.mult)

                    if no_mm:
                        continue
                    for kk in range(KSUP):
                        q = sup * KSUP + kk
                        t = int(tile_of_chunk[q])
                        if q == first_chunk[t]:
                            ps_cur = psum_seg.tile([128, GW], dt.float32,
                                                   tag="segps")
                        nc.tensor.matmul(
                            ps_cur[:, :gw],
                            lhsT=mt[:, kk * 128:(kk + 1) * 128],
                            rhs=gt[:, kk, :gw],
                            start=(q == first_chunk[t]),
                            stop=(q == last_chunk[t]))
                        if q == last_chunk[t]:
                            pending.append((t, ps_cur))
                for (t, ps) in pending:
                    run_epilogue(t, ps)

            for _rep in range(repeat):
                tab1 = dram.tile([NCORES * nmax, TW], dt.bfloat16,
                                 name=f"tab1_{_rep}")
                tab2 = dram.tile([NCORES * nmax, TW], dt.bfloat16,
                                 addr_space="Shared", name=f"tab2_{_rep}")
                tab3 = dram.tile([NCORES * nmax, TW3], dt.bfloat16,
                                 addr_space="Shared", name=f"tab3_{_rep}")
                h1_phase(tab1)
                if nphase >= 1:
                    aggregate(1, tab1, adl1, rhs2_s, b1_s)
                if nphase >= 2:
                    flush_adst(adst_sb2, adl2, HEADS)
                    allgather(loc12, tab2)
                    aggregate(2, tab2, adl2, rhs3_s, b2_s)
                if nphase >= 3:
                    flush_adst(adst_sb3, adl3, 1)
                    allgather(loc3, tab3)
                    aggregate(3, tab3, adl3, None, None)

    nc.compile()
    return nc


def _make_in_maps(x, g, wts):
    """Per-core input dicts. x: [N, IN_C] f32; wts: _prep_weights output."""
    rhs1, rhs1d, rhs2, rhs3, b1r, b2r, b3r = wts
    npc, nmax = g["npc"], g["nmax"]
    iota = np.tile(np.arange(128, dtype=np.float32)[None, :],
                   (128, 1)).astype(_BF16)
    xTf = np.zeros((IN_C, NCORES * nmax), _BF16)
    for k in range(NCORES):
        xTf[:, k * nmax:k * nmax + npc] = x[k * npc:(k + 1) * npc].T
    in_maps = []
    for k in range(NCORES):
        in_maps.append({
            "xT": xTf, "x_own": xTf[:, k * nmax:(k + 1) * nmax],
            "rhs1": rhs1, "rhs1d": rhs1d, "rhs2": rhs2, "rhs3": rhs3,
            "b1r": b1r, "b2r": b2r, "b3r": b3r, "iota": iota,
            "idxs": np.ascontiguousarray(g["idxc"][k]).reshape(
                128, -1),
        })
    return in_maps


_CACHE = {}


def kernel(x, edge_index, W1, as1, ad1, b1, W2, as2, ad2, b2, W3, as3, ad3, b3,
           _repeat=1):
    from concourse.bass_utils import run_bass_kernel_spmd

    x = np.asarray(x, np.float32)
    edge_index = np.asarray(edge_index)
    g = _prep_graph(edge_index)
    wts = _prep_weights(W1, as1, ad1, b1, W2, as2, ad2, b2, W3, as3, ad3, b3)

    key = (hash(edge_index.tobytes()), _repeat)
    if key not in _CACHE:
        _CACHE[key] = _build_bass(g, repeat=_repeat)
    nc = _CACHE[key]

    in_maps = _make_in_maps(x, g, wts)
    res = run_bass_kernel_spmd(nc, in_maps, core_ids=list(range(NCORES)))
    npc = g["npc"]
    outf = np.zeros((N, OUT_C), np.float32)
    for k in range(NCORES):
        outf[k * npc:(k + 1) * npc] = res.results[k]["out"][:npc]
    return outf
